# revision 43
# baseline (speedup 1.0000x reference)
"""Trainium2 Bass kernel for nn_LorentzGNN (2x GATv2 + Lorentz head), 8-core SPMD.

Sharding: nodes (and their in-edges) are partitioned contiguously across 8 cores
(2048 nodes each). Each core computes its shard's GAT transforms; the xl source
table is replicated via chunked AllGathers (Shared addr space) overlapped with
the transform matmuls. Per-edge work uses a single edge-major dma_gather plus
host-built 0/1 segment matrices fed to the PE as matmuls: xr[dst] broadcast is
a segment matmul, leaky-relu runs on the scalar engine (AF.Lrelu), attention
logits are fused multiply+reduce ops on the vector engine, and softmax
denominator + weighted aggregation are segment matmuls. Graph-level ops
(centroid accumulation, Lorentz MLP over the 16 graphs each core owns) are
fused into the layer-2 epilogue; host concatenates per-core outputs.
"""
import numpy as np
import ml_dtypes

# ---------- problem constants (hardcoded per contract) ----------
N, E, B = 16384, 131072, 128
FT, HEADS, C = 512, 4, 128
NCORES = 8
SHARD = N // NCORES            # 2048
P = 128
NT = SHARD // P                # 16 dst tiles per core
GPC = B // NCORES              # 16 graphs per core
LEAK = 0.2
NCHUNK = 8                     # AllGather chunks per layer

_cache = {}


# ============================ host-side prep ============================

def _prep_edges(edge_index):
    src = np.concatenate([edge_index[0], np.arange(N)]).astype(np.int64)
    dst = np.concatenate([edge_index[1], np.arange(N)]).astype(np.int64)
    # remap src ids to the chunk-major AllGathered table layout:
    # node n = k*SHARD + g*CR + r lives at table row g*(8*CR) + k*CR + r
    CR = SHARD // NCHUNK
    src = (src % SHARD // CR) * (NCORES * CR) + (src // SHARD) * CR + src % CR
    order = np.argsort(dst, kind="stable")
    src, dst = src[order], dst[order]
    ntiles = N // P
    counts = np.bincount(dst // P, minlength=ntiles)
    LP = int(np.ceil(max(counts.max(), 1) / 128) * 128)
    NJ = LP // P
    srcs = np.zeros((ntiles, LP), np.int16)
    dloc = np.full((ntiles, LP), -1, np.int32)
    starts = np.concatenate([[0], np.cumsum(counts)])
    # per tile-slot block count: max over the 8 cores sharing that slot
    nt = ntiles // NCORES
    cmax = counts.reshape(NCORES, nt).max(0)
    njt = [int(np.ceil(max(c, 1) / 128)) for c in cmax]
    for t in range(ntiles):
        c = counts[t]
        srcs[t, :c] = src[starts[t]:starts[t] + c]
        dloc[t, :c] = dst[starts[t]:starts[t] + c] - t * P
    # segment matrices
    sdt = np.zeros((ntiles, P, LP), np.float32)          # [dst, j]
    jj = np.arange(LP)
    for t in range(ntiles):
        v = dloc[t] >= 0
        sdt[t, dloc[t, v], jj[v]] = 1.0
    sjt = np.ascontiguousarray(sdt.transpose(0, 2, 1))   # [j, dst] edge-major
    # idx buffers wrapped in 16 partitions, replicated to 128
    idx = srcs.reshape(ntiles, LP // 16, 16).transpose(0, 2, 1)  # [t, 16, LP/16]
    idx = np.tile(idx, (1, 8, 1)).astype(np.int16)               # [t, 128, LP/16]
    return srcs, sdt, sjt, idx, LP, NJ, njt


def _interleave_k(w, kchunks):
    """[K*128, N] -> [128, K, N] -> host layout [128, K*N] for SBUF."""
    K, Nn = w.shape
    assert K == kchunks * 128
    return np.ascontiguousarray(w.reshape(kchunks, 128, Nn).transpose(1, 0, 2))


def _aug_w(W, b, kpad, npad=None):
    """stack rows [W; b; 0-pad] to kpad rows, optionally pad cols to npad."""
    K, Nn = W.shape
    out = np.zeros((kpad, Nn if npad is None else npad), np.float32)
    out[:K, :Nn] = W
    out[K, :Nn] = b
    return out


# ============================ kernel build ============================

def _build(LP, NJ, ex_dtype_name, njt=None):
    njt = list(njt) if njt is not None else [NJ] * NT
    import concourse.bass as bass
    import concourse.bacc as bacc
    import concourse.tile as tile
    from concourse import mybir
    from concourse.library_config import mlp as gpsimd_mlp

    f32, f16 = mybir.dt.float32, mybir.dt.float16
    bf16, i16 = mybir.dt.bfloat16, mybir.dt.int16
    EXD = {"float16": f16, "bfloat16": bf16}[ex_dtype_name]
    AF = mybir.ActivationFunctionType
    ALU = mybir.AluOpType
    SCT = [[(o, min(512, nj * 128 - o)) for o in range(0, nj * 128, 512)]
           for nj in njt]                 # per-slot gather slices
    NREG = sorted({n for sc in SCT for _, n in sc})
    CR = SHARD // NCHUNK      # rows per AllGather chunk (512)

    nc = bacc.Bacc("TRN2", target_bir_lowering=False, debug=False,
                   num_devices=NCORES)
    groups = [list(range(NCORES))]

    # ---- DRAM I/O (per-core, same program) ----
    xTa = nc.dram_tensor("xTa", [128, 5 * SHARD], f16, kind="ExternalInput")
    w1l = nc.dram_tensor("w1l", [128, 5 * FT], f16, kind="ExternalInput")
    w1r = nc.dram_tensor("w1r", [128, 5 * FT], f16, kind="ExternalInput")
    w2l = nc.dram_tensor("w2l", [128, 4 * FT], f16, kind="ExternalInput")
    w2r = nc.dram_tensor("w2r", [128, 4 * FT], f16, kind="ExternalInput")
    b2r_d = nc.dram_tensor("b2rows", [2, FT], f16, kind="ExternalInput")
    af1_d = nc.dram_tensor("attf1", [128, FT], f16, kind="ExternalInput")
    af2_d = nc.dram_tensor("attf2", [128, FT], f16, kind="ExternalInput")
    b1f_d = nc.dram_tensor("b1full", [128, FT], f32, kind="ExternalInput")
    b2f_d = nc.dram_tensor("b2full", [128, FT], f32, kind="ExternalInput")
    sdt_d = nc.dram_tensor("sdt", [NT, 128, LP], f16, kind="ExternalInput")
    sj_d = nc.dram_tensor("sj", [NT, 128, NJ * 128], EXD, kind="ExternalInput")
    idx_d = nc.dram_tensor("idx", [NT, 128, LP // 16], i16, kind="ExternalInput")
    ecols_d = nc.dram_tensor("ecols", [128, NT * GPC], f16, kind="ExternalInput")
    ident_d = nc.dram_tensor("ident", [128, 128], f16, kind="ExternalInput")
    wa_d = nc.dram_tensor("wa", [128, 5 * 2560], f16, kind="ExternalInput")
    wb_d = nc.dram_tensor("wb", [128, 17 * 1024], f16, kind="ExternalInput")
    wf_d = nc.dram_tensor("wf", [128, 5 * 640], f16, kind="ExternalInput")
    sabf_d = nc.dram_tensor("sabf", [16, 3], f32, kind="ExternalInput")  # sa,sb,sf

    xl1_sh = nc.dram_tensor("xl1_sh", [SHARD, FT], f16)
    xl2_sh = nc.dram_tensor("xl2_sh", [SHARD, FT], f16)
    xl1_tb = nc.dram_tensor("xl1_tb", [N, FT], f16, addr_space="Shared")
    xl2_tb = nc.dram_tensor("xl2_tb", [N, FT], f16, addr_space="Shared")
    zout = nc.dram_tensor("zout", [GPC, FT + 1], f32, kind="ExternalOutput")
    gmout = nc.dram_tensor("gmout", [GPC, FT + 1], f32, kind="ExternalOutput")

    with tile.TileContext(nc, num_cores=NCORES) as tc:
        import contextlib
        est = contextlib.ExitStack()
        with est:
            nc.gpsimd.load_library(gpsimd_mlp)
            nregs = {n: nc.gpsimd.to_reg(n) for n in NREG}
            cpool = est.enter_context(tc.tile_pool(name="consts", bufs=1))
            wpool = est.enter_context(tc.tile_pool(name="wmlp", bufs=1))
            xrp = est.enter_context(tc.tile_pool(name="xr", bufs=1))
            h1p = est.enter_context(tc.tile_pool(name="h1", bufs=1))
            sbp = est.enter_context(tc.tile_pool(name="stream", bufs=2))
            smp = est.enter_context(tc.tile_pool(name="small", bufs=2))
            msb = est.enter_context(tc.tile_pool(name="mstream", bufs=2))
            psb = est.enter_context(tc.tile_pool(name="psb", bufs=4, space="PSUM"))
            pss = est.enter_context(tc.tile_pool(name="pss", bufs=2, space="PSUM"))
            pgmp = est.enter_context(tc.tile_pool(name="pgm", bufs=1, space="PSUM"))

            # ---- consts ----
            w1l_s = cpool.tile([128, 5 * FT], f16, name="w1l_s")
            nc.sync.dma_start(w1l_s[:], w1l[:])
            w1r_s = cpool.tile([128, 5 * FT], f16, name="w1r_s")
            nc.sync.dma_start(w1r_s[:], w1r[:])
            w2l_s = cpool.tile([128, 4 * FT], f16, name="w2l_s")
            nc.sync.dma_start(w2l_s[:], w2l[:])
            w2r_s = cpool.tile([128, 4 * FT], f16, name="w2r_s")
            nc.sync.dma_start(w2r_s[:], w2r[:])
            b2la_s = cpool.tile([1, FT], f16, name="b2la_s")
            nc.sync.dma_start(b2la_s[:], b2r_d[0:1, :])
            b2ra_s = cpool.tile([1, FT], f16, name="b2ra_s")
            nc.sync.dma_start(b2ra_s[:], b2r_d[1:2, :])
            af1_s = cpool.tile([128, FT], f16, name="af1_s")
            nc.sync.dma_start(af1_s[:], af1_d[:])
            af2_s = cpool.tile([128, FT], f16, name="af2_s")
            nc.sync.dma_start(af2_s[:], af2_d[:])
            b1f_s = cpool.tile([128, FT], f32, name="b1f_s")
            nc.sync.dma_start(b1f_s[:], b1f_d[:])
            b2f_s = cpool.tile([128, FT], f32, name="b2f_s")
            nc.sync.dma_start(b2f_s[:], b2f_d[:])
            ident_s = cpool.tile([128, 128], f16, name="ident_s")
            nc.sync.dma_start(ident_s[:], ident_d[:])
            ecols_s = cpool.tile([128, NT * GPC], f16, name="ecols_s")
            nc.sync.dma_start(ecols_s[:], ecols_d[:])
            ones1 = cpool.tile([1, FT], f16, name="ones1")
            nc.vector.memset(ones1[:], 1.0)
            sabf_s = cpool.tile([16, 3], f32, name="sabf_s")
            nc.sync.dma_start(sabf_s[:], sabf_d[:])
            esc = cpool.tile([16, 3], f32, name="esc")
            nc.scalar.activation(esc[:], sabf_s[:], AF.Exp)
            z0p = cpool.tile([16, 640], f16, name="z0p")
            nc.vector.memset(z0p[:], 0.0)
            nc.vector.memset(z0p[:, 513:514], 1.0)
            onescol = cpool.tile([128, 1], f16, name="onescol")
            nc.vector.memset(onescol[:], 1.0)
            tacc = cpool.tile([128, NT], f32, name="tacc")
            alph = cpool.tile([128, 1], f32, name="alph")
            nc.vector.memset(alph[:], LEAK)
            nege = cpool.tile([128, 1], f32, name="nege")
            nc.vector.memset(nege[:], -2.0)
            # MLP weights rotate through one 35KB buffer: wa early (overlaps
            # the GNN), wb/wf reload behind each llin stage.
            wa_s = wpool.tile([128, 17 * 1024], f16, tag="w", name="wa_s", bufs=1)
            nc.sync.dma_start(wa_s[:, 0:5 * 2560], wa_d[:])
            wa_v = wa_s[:, 0:5 * 2560].rearrange("p (k n) -> p k n", k=5)

            def nsqrt(out_ap, x_ap, pool, pfx):
                """out = sqrt(x), Newton-refined (ACT sqrt LUT is ~4e-3)."""
                y0 = pool.tile(list(x_ap.shape), f32, tag="nsq", name=pfx + "y0",
                               bufs=6)
                nc.scalar.activation(y0[:], x_ap, AF.Sqrt)
                r0 = pool.tile(list(x_ap.shape), f32, tag="nsq", name=pfx + "r0",
                               bufs=6)
                nc.vector.reciprocal(r0[:], y0[:])
                nc.vector.tensor_tensor(out=r0[:], in0=x_ap, in1=r0[:],
                                        op=ALU.mult)
                nc.vector.tensor_tensor(out=y0[:], in0=y0[:], in1=r0[:],
                                        op=ALU.add)
                nc.vector.tensor_scalar_mul(out_ap, y0[:], 0.5)

            xr_s = xrp.tile([128, NT * FT], f16, name="xr_s")       # resident xr
            h1_s = h1p.tile([128, NT * FT], f16, name="h1_s")       # resident h1
            h1pre = h1p.tile([128, NT * FT], f16, name="h1pre")     # pre-gelu

            def allgather(g, sh, tb):
                # table rows are chunk-major (g, k, r): each chunk's gathered
                # output is one contiguous [8*CR, FT] slice (BIR requires it)
                nc.gpsimd.collective_compute(
                    "AllGather", ALU.bypass, replica_groups=groups,
                    ins=[sh[g * CR:(g + 1) * CR, :]],
                    outs=[tb[g * NCORES * CR:(g + 1) * NCORES * CR, :]])

            def transform1():
                xTa_v = xTa[:].rearrange("p (k n) -> p k n", k=5)
                w1l_v = w1l_s[:].rearrange("p (k n) -> p k n", k=5)
                w1r_v = w1r_s[:].rearrange("p (k n) -> p k n", k=5)
                for t in range(NT):
                    xt = smp.tile([128, 5 * 128], f16, tag="xTa_t", name="xt",
                                  bufs=3)
                    nc.sync.dma_start(
                        xt[:].rearrange("p (k n) -> p k n", k=5),
                        xTa_v[:, :, t * 128:(t + 1) * 128])
                    xt_v = xt[:].rearrange("p (k n) -> p k n", k=5)
                    pl = psb.tile([128, FT], f32, tag="pbig", name="pl")
                    pr = psb.tile([128, FT], f32, tag="pbig", name="pr")
                    for kc in range(5):
                        nc.tensor.matmul(pl[:], lhsT=xt_v[:, kc, :],
                                         rhs=w1l_v[:, kc, :],
                                         start=(kc == 0), stop=(kc == 4))
                        nc.tensor.matmul(pr[:], lhsT=xt_v[:, kc, :],
                                         rhs=w1r_v[:, kc, :],
                                         start=(kc == 0), stop=(kc == 4))
                    xlt = smp.tile([128, FT], f16, tag="xlt", name="xlt")
                    nc.scalar.activation(xlt[:], pl[:], AF.Copy)
                    nc.scalar.activation(xr_s[:, t * FT:(t + 1) * FT], pr[:],
                                         AF.Copy)
                    nc.sync.dma_start(xl1_sh[t * 128:(t + 1) * 128, :], xlt[:])
                    if (t + 1) % (NT // NCHUNK) == 0:
                        allgather(t // (NT // NCHUNK), xl1_sh, xl1_tb)

            def transform2():
                w2l_v = w2l_s[:].rearrange("p (k n) -> p k n", k=4)
                w2r_v = w2r_s[:].rearrange("p (k n) -> p k n", k=4)
                for t in range(NT):
                    h1t = h1_s[:].rearrange("p (t n) -> p t n", t=NT)[:, t, :]
                    h1T = smp.tile([128, 4 * 128], f16, tag="h1T", name="h1T")
                    for fc in range(4):
                        pt = pss.tile([128, 128], f16, tag="pe", name="pt",
                                      bufs=1)
                        nc.tensor.transpose(pt[:], h1t[:, fc * 128:(fc + 1) * 128],
                                            ident_s[:])
                        nc.scalar.activation(h1T[:, fc * 128:(fc + 1) * 128],
                                             pt[:], AF.Copy)
                    pl = psb.tile([128, FT], f32, tag="pbig", name="pl2")
                    pr = psb.tile([128, FT], f32, tag="pbig", name="pr2")
                    h1T_v = h1T[:].rearrange("p (k n) -> p k n", k=4)
                    for kc in range(4):
                        nc.tensor.matmul(pl[:], lhsT=h1T_v[:, kc, :],
                                         rhs=w2l_v[:, kc, :],
                                         start=(kc == 0), stop=False)
                        nc.tensor.matmul(pr[:], lhsT=h1T_v[:, kc, :],
                                         rhs=w2r_v[:, kc, :],
                                         start=(kc == 0), stop=False)
                    nc.tensor.matmul(pl[:], lhsT=ones1[:, 0:128], rhs=b2la_s[:],
                                     start=False, stop=True)
                    nc.tensor.matmul(pr[:], lhsT=ones1[:, 0:128], rhs=b2ra_s[:],
                                     start=False, stop=True)
                    xlt = smp.tile([128, FT], f16, tag="xlt", name="xlt2")
                    nc.scalar.activation(xlt[:], pl[:], AF.Copy)
                    nc.scalar.activation(xr_s[:, t * FT:(t + 1) * FT], pr[:],
                                         AF.Copy)
                    nc.sync.dma_start(xl2_sh[t * 128:(t + 1) * 128, :], xlt[:])
                    if (t + 1) % (NT // NCHUNK) == 0:
                        allgather(t // (NT // NCHUNK), xl2_sh, xl2_tb)

            def edge_layer(layer, table, af_s):
                """GATv2 message passing; writes h1_s (layer1) or, for layer2,
                the h2 epilogue + centroid accumulation + z0 extraction."""
                if layer == 2:
                    pgm = pgmp.tile([128, 640], f32, name="pgm")
                for t in range(NT):
                    NJt = njt[t]
                    idxt = smp.tile([128, LP // 16], i16, tag="idxt",
                                    name="idxt", bufs=4)
                    nc.sync.dma_start(idxt[:, 0:NJt * 8],
                                      idx_d[t, :, 0:NJt * 8])
                    sdtt = sbp.tile([128, LP], f16, tag="sdtt", name="sdtt")
                    nc.sync.dma_start(sdtt[:, 0:NJt * 128],
                                      sdt_d[t, :, 0:NJt * 128])
                    sjt = sbp.tile([128, NJ * 128], EXD, tag="sjt", name="sjt")
                    nc.sync.dma_start(sjt[:, 0:NJt * 128],
                                      sj_d[t, :, 0:NJt * 128])
                    sj_v = sjt[:].rearrange("p (j d) -> p j d", j=NJ)

                    xlg = sbp.tile([128, NJ * FT], f16, tag="xlg", name="xlg")
                    xlg_w = xlg[:].rearrange("p (j n) -> p j n", j=NJ)
                    for (o, n) in SCT[t]:
                        nc.gpsimd.dma_gather(
                            xlg_w[:, o // 128:(o + n) // 128, :], table[:],
                            idxt[:, o // 16:(o + n) // 16], n, nregs[n], FT)
                    xlg_v = xlg[:].rearrange("p (j n) -> p j n", j=NJ)

                    xr_t = xr_s[:].rearrange("p (t n) -> p t n", t=NT)[:, t, :]
                    logit = smp.tile([128, NJ * HEADS], f32, tag="logit",
                                     name="logit")
                    for jb in range(NJt):
                        ps2 = psb.tile([128, FT], f32, tag="pbig", name="ps2")
                        nc.tensor.matmul(ps2[:],
                                         lhsT=sdtt[:, jb * 128:(jb + 1) * 128],
                                         rhs=xr_t, start=True, stop=False)
                        nc.tensor.matmul(ps2[:], lhsT=ident_s[:],
                                         rhs=xlg_v[:, jb, :],
                                         start=False, stop=True)
                        lr = smp.tile([128, FT], f16, tag="lr", name="lr",
                                      bufs=3)
                        nc.scalar.activation(lr[:], ps2[:], AF.Prelu,
                                             alpha=alph[:])
                        scr = smp.tile([128, FT], f16, tag="scr", name="scr",
                                       bufs=2)
                        for h in range(HEADS):
                            nc.vector.scalar_tensor_tensor(
                                out=scr[:, h * C:(h + 1) * C],
                                in0=lr[:, h * C:(h + 1) * C], scalar=1.0,
                                in1=af_s[:, h * C:(h + 1) * C],
                                op0=ALU.mult, op1=ALU.mult,
                                accum_out=logit[:, jb * HEADS + h:
                                                jb * HEADS + h + 1])
                    exf = smp.tile([128, NJ * HEADS], f32, tag="exf", name="exf")
                    nc.scalar.activation(exf[:, 0:NJt * HEADS],
                                         logit[:, 0:NJt * HEADS], AF.Exp,
                                         bias=nege[:])
                    ex = smp.tile([128, NJ * HEADS], EXD, tag="ex", name="ex")
                    nc.scalar.activation(ex[:, 0:NJt * HEADS],
                                         exf[:, 0:NJt * HEADS], AF.Copy)
                    ex_v = ex[:].rearrange("p (j h) -> p j h", j=NJ)
                    pden = pss.tile([128, HEADS], f32, tag="pden", name="pden",
                                    bufs=1)
                    pagg = psb.tile([128, FT], f32, tag="pbig", name="pagg")
                    for jb in range(NJt):
                        wt = smp.tile([128, FT], EXD, tag="wt", name="wt",
                                      bufs=3)
                        nc.vector.tensor_tensor(
                            out=wt[:].rearrange("p (h c) -> p h c", h=HEADS),
                            in0=xlg_v[:, jb, :].rearrange("p (h c) -> p h c",
                                                          h=HEADS),
                            in1=ex_v[:, jb, :].broadcast_to([128, HEADS, C]),
                            op=ALU.mult)
                        nc.tensor.matmul(pden[:], lhsT=sj_v[:, jb, :],
                                         rhs=ex_v[:, jb, :],
                                         start=(jb == 0), stop=(jb == NJt - 1))
                        nc.tensor.matmul(pagg[:], lhsT=sj_v[:, jb, :], rhs=wt[:],
                                         start=(jb == 0), stop=(jb == NJt - 1))
                    rden = smp.tile([128, HEADS], f32, tag="rden", name="rden")
                    nc.vector.reciprocal(rden[:], pden[:])
                    # epilogue: out = pagg*rden (per head) + bias (on gpsimd)
                    if layer == 1:
                        for h in range(HEADS):
                            nc.vector.scalar_tensor_tensor(
                                out=h1pre[:, t * FT + h * C:t * FT + (h + 1) * C],
                                in0=pagg[:, h * C:(h + 1) * C],
                                scalar=rden[:, h:h + 1],
                                in1=b1f_s[:, h * C:(h + 1) * C],
                                op0=ALU.mult, op1=ALU.add)
                        if (t + 1) % (NT // NCHUNK) == 0:
                            g = t // (NT // NCHUNK)
                            for tg in range(g * (NT // NCHUNK), (g + 1) *
                                            (NT // NCHUNK)):
                                nc.scalar.activation(
                                    h1_s[:, tg * FT:(tg + 1) * FT],
                                    h1pre[:, tg * FT:(tg + 1) * FT], AF.Gelu)
                    else:
                        h2sp = smp.tile([128, FT], f32, tag="h2sp", name="h2sp")
                        for h in range(HEADS):
                            nc.vector.scalar_tensor_tensor(
                                out=h2sp[:, h * C:(h + 1) * C],
                                in0=pagg[:, h * C:(h + 1) * C],
                                scalar=rden[:, h:h + 1],
                                in1=b2f_s[:, h * C:(h + 1) * C],
                                op0=ALU.mult, op1=ALU.add)
                        sqj = smp.tile([128, FT], f16, tag="sqj", name="sqj")
                        nc.vector.scalar_tensor_tensor(
                            out=sqj[:], in0=h2sp[:], scalar=1.0, in1=h2sp[:],
                            op0=ALU.mult, op1=ALU.mult,
                            accum_out=tacc[:, t:t + 1])
                        h2c = smp.tile([128, FT], f16, tag="h2c", name="h2c")
                        nc.scalar.activation(h2c[:], h2sp[:], AF.Copy)
                        ec = ecols_s[:, t * GPC:(t + 1) * GPC]
                        nc.tensor.matmul(pgm[:GPC, 0:FT], lhsT=ec, rhs=h2c[:],
                                         start=(t == 0), stop=(t == NT - 1))
                        nc.sync.dma_start(z0p[t:t + 1, 1:FT + 1], h2c[0:1, :])
                if layer == 2:
                    # batched time coordinate: t = sqrt(1 + |s|^2) for all tiles
                    nc.vector.tensor_scalar_add(tacc[:], tacc[:], 1.0)
                    tsq = smp.tile([128, NT], f32, tag="tsq", name="tsq")
                    nsqrt(tsq[:], tacc[:], smp, "t_")
                    tc16 = smp.tile([128, NT], f16, tag="tc16", name="tc16")
                    nc.vector.tensor_copy(tc16[:], tsq[:])
                    nc.tensor.matmul(pgm[:GPC, FT:FT + 1], lhsT=tc16[:],
                                     rhs=onescol[:], start=True, stop=True)
                    # z0 time coord recomputed from its (f16) space part
                    zsqj = msb.tile([16, FT], f16, tag="zsqj", name="zsqj")
                    zta = msb.tile([16, 1], f32, tag="t1", name="zta", bufs=8)
                    nc.vector.scalar_tensor_tensor(
                        out=zsqj[:], in0=z0p[:, 1:FT + 1], scalar=1.0,
                        in1=z0p[:, 1:FT + 1], op0=ALU.mult, op1=ALU.mult,
                        accum_out=zta[:])
                    nc.vector.tensor_scalar_add(zta[:], zta[:], 1.0)
                    ztb = msb.tile([16, 1], f32, tag="t1", name="ztb", bufs=8)
                    nsqrt(ztb[:], zta[:], msb, "zt_")
                    nc.scalar.activation(z0p[:, 0:1], ztb[:], AF.Copy)
                    return pgm

            transform1()
            edge_layer(1, xl1_tb, af1_s)
            transform2()
            pgm = edge_layer(2, xl2_tb, af2_s)

            # -------- centroid epilogue (pgm layout: [space(512) | time]) ----
            sums = smp.tile([GPC, FT + 1], f32, tag="sums", name="sums")
            nc.scalar.activation(sums[:], pgm[:GPC, 0:FT + 1], AF.Copy)
            sqgj = smp.tile([GPC, FT], f16, tag="sqgj", name="sqgj")
            sa_ = smp.tile([GPC, 1], f32, tag="sacc", name="sa_")
            nc.vector.scalar_tensor_tensor(
                out=sqgj[:], in0=sums[:, 0:FT], scalar=1.0, in1=sums[:, 0:FT],
                op0=ALU.mult, op1=ALU.mult, accum_out=sa_[:])
            innr = smp.tile([GPC, 1], f32, tag="in1", name="innr")
            nc.vector.tensor_tensor(out=innr[:], in0=sums[:, FT:FT + 1],
                                    in1=sums[:, FT:FT + 1], op=ALU.mult)
            nc.vector.tensor_tensor(out=innr[:], in0=innr[:], in1=sa_[:],
                                    op=ALU.subtract)
            nc.vector.tensor_scalar_max(innr[:], innr[:], 1e-8 * (N // B) ** 2)
            rt = smp.tile([GPC, 1], f32, tag="in1", name="rt")
            nsqrt(rt[:], innr[:], smp, "g_")
            nc.vector.reciprocal(rt[:], rt[:])
            gmt = smp.tile([GPC, FT + 1], f32, tag="sums", name="gmt")
            nc.scalar.activation(gmt[:, 0:1], sums[:, FT:FT + 1], AF.Copy,
                                 scale=rt[:])
            nc.scalar.activation(gmt[:, 1:FT + 1], sums[:, 0:FT], AF.Copy,
                                 scale=rt[:])
            nc.sync.dma_start(gmout[:], gmt[:])

            # ---------------- Lorentz MLP on z0 [16, 513] ----------------
            def trans_blocks(zp, kb):
                """zp [16, kb*128] f16 -> zT [128, kb*16] f16 via PE."""
                zT = msb.tile([128, 17 * 16], f16, tag="zT", name="zT")
                for k in range(kb):
                    pt = pss.tile([128, 128], f16, tag="pe", name="ptm",
                                  bufs=1)
                    nc.tensor.transpose(pt[:, 0:16], zp[:, k * 128:(k + 1) * 128],
                                        ident_s[:16, :16])
                    nc.scalar.activation(zT[:, k * 16:(k + 1) * 16], pt[:, 0:16],
                                         AF.Copy)
                return zT

            def mm_thin(zT, kb, w_v, ncols):
                """out [16, ncols] f32 = zT.T @ w; w_v view [128, kb, ncols]."""
                out = msb.tile([16, 2560], f32, tag="mlpo", name="out", bufs=1)
                zT_v = zT[:].rearrange("p (k n) -> p k n", k=17)
                for o in range(0, ncols, 512):
                    n = min(512, ncols - o)
                    pm = psb.tile([128, FT], f32, tag="pbig", name="pm")
                    for k in range(kb):
                        nc.tensor.matmul(pm[:16, :n], lhsT=zT_v[:, k, :16],
                                         rhs=w_v[:, k, o:o + n],
                                         start=(k == 0), stop=(k == kb - 1))
                    nc.scalar.activation(out[:, o:o + n], pm[:16, :n], AF.Copy)
                return out

            def llin_post(zz, kout, esc_idx):
                """returns (t1, r_) for zz [16, ncols>=kout] f32."""
                t1 = msb.tile([16, 1], f32, tag="t1", name="t1", bufs=8)
                nc.scalar.activation(t1[:], zz[:, 0:1], AF.Sigmoid)
                nc.vector.tensor_scalar(
                    out=t1[:], in0=t1[:],
                    scalar1=esc[:, esc_idx:esc_idx + 1],
                    scalar2=1.1, op0=ALU.mult, op1=ALU.add)
                sq = msb.tile([16, 2048], f32, tag="msq", name="sq", bufs=1)
                ac = msb.tile([16, 1], f32, tag="t1", name="ac", bufs=8)
                nc.scalar.activation(sq[:, :kout - 1], zz[:, 1:kout], AF.Square,
                                     accum_out=ac[:])
                nc.vector.tensor_scalar_max(ac[:], ac[:], 1e-8)
                r_ = msb.tile([16, 1], f32, tag="t1", name="r_", bufs=8)
                nc.vector.reciprocal(r_[:], ac[:])
                t2 = msb.tile([16, 1], f32, tag="t1", name="t2", bufs=8)
                nc.vector.tensor_tensor(out=t2[:], in0=t1[:], in1=t1[:],
                                        op=ALU.mult)
                nc.vector.tensor_scalar_add(t2[:], t2[:], -1.0)
                nc.vector.tensor_tensor(out=r_[:], in0=r_[:], in1=t2[:],
                                        op=ALU.mult)
                nsqrt(r_[:], r_[:], msb, "m_")
                return t1, r_

            # llin-a: z0p [16, 640] -> zA [16, 2560]
            zT = trans_blocks(z0p, 5)
            zA = mm_thin(zT, 5, wa_v, 2560)
            t1, r1 = llin_post(zA, 2049, 0)
            wb_s = wpool.tile([128, 17 * 1024], f16, tag="w", name="wb_s",
                              bufs=1)
            nc.sync.dma_start(wb_s[:], wb_d[:])
            wb_v = wb_s[:].rearrange("p (k n) -> p k n", k=17)
            # z1 = add_time(gelu(sp*r1)): gelu with scale=r1
            z1p = msb.tile([16, 17 * 128], f16, tag="z1p", name="z1p", bufs=1)
            nc.vector.memset(z1p[:], 0.0)
            nc.scalar.activation(z1p[:, 1:2049], zA[:, 1:2049], AF.Gelu,
                                 scale=r1[:])
            sqz = msb.tile([16, 2048], f32, tag="msq", name="sqz", bufs=1)
            az = msb.tile([16, 1], f32, tag="t1", name="az", bufs=8)
            nc.scalar.activation(sqz[:], z1p[:, 1:2049], AF.Square,
                                 accum_out=az[:])
            az1 = msb.tile([16, 1], f32, tag="t1", name="az1", bufs=8)
            nc.scalar.activation(az1[:], az[:], AF.Identity, bias=1.0)
            nsqrt(z1p[:, 0:1], az1[:], msb, "z_")
            nc.vector.memset(z1p[:, 2049:2050], 1.0)
            # llin-b: [16, 2049] -> [16, 513]
            zTb = trans_blocks(z1p, 17)
            zB = mm_thin(zTb, 17, wb_v, 1024)
            t3, r3 = llin_post(zB, 513, 1)
            wf_s = wpool.tile([128, 17 * 1024], f16, tag="w", name="wf_s",
                              bufs=1)
            nc.sync.dma_start(wf_s[:, 0:5 * 640], wf_d[:])
            wf_v = wf_s[:, 0:5 * 640].rearrange("p (k n) -> p k n", k=5)
            z2p = msb.tile([16, 640], f16, tag="z2p", name="z2p", bufs=1)
            nc.vector.memset(z2p[:], 0.0)
            nc.scalar.activation(z2p[:, 0:1], t3[:], AF.Copy)
            nc.scalar.activation(z2p[:, 1:513], zB[:, 1:513], AF.Copy,
                                 scale=r3[:])
            nc.vector.memset(z2p[:, 513:514], 1.0)
            # llin-f: [16, 513] -> [16, 513]
            zTf = trans_blocks(z2p, 5)
            zF = mm_thin(zTf, 5, wf_v, 640)
            t4, r4 = llin_post(zF, 513, 2)
            zfin = msb.tile([16, 640], f32, tag="zfin", name="zfin", bufs=1)
            nc.scalar.activation(zfin[:, 0:1], t4[:], AF.Copy)
            nc.scalar.activation(zfin[:, 1:513], zF[:, 1:513], AF.Copy,
                                 scale=r4[:])
            nc.sync.dma_start(zout[:], zfin[:, 0:FT + 1])

    nc.compile()
    return nc


# ============================ host entry ============================

EX_DTYPE = "float16"    # logits are small; exp shifted by -2


def _make_inmaps(inputs):
    x = np.asarray(inputs["x"], np.float32)
    edge_index = np.asarray(inputs["edge_index"])
    srcs, sdt, sjt, idx, LP, NJ, njt = _prep_edges(edge_index)

    f16 = np.float16
    exd_np = ml_dtypes.bfloat16 if EX_DTYPE == "bfloat16" else np.float16

    # ---- shared (replicated) host arrays ----
    def aug5(W, b):
        return _interleave_k(_aug_w(np.asarray(W, np.float32),
                                    np.asarray(b, np.float32), 640), 5)

    w1l_h = aug5(inputs["Wl1"], inputs["bl1"]).astype(f16).reshape(128, 5 * FT)
    w1r_h = aug5(inputs["Wr1"], inputs["br1"]).astype(f16).reshape(128, 5 * FT)
    w2l_h = _interleave_k(np.asarray(inputs["Wl2"], np.float32), 4
                          ).astype(f16).reshape(128, 4 * FT)
    w2r_h = _interleave_k(np.asarray(inputs["Wr2"], np.float32), 4
                          ).astype(f16).reshape(128, 4 * FT)
    b2rows = np.stack([np.asarray(inputs["bl2"]),
                       np.asarray(inputs["br2"])]).astype(f16)

    def amat(att):
        att = np.asarray(att, np.float32).reshape(1, HEADS * C)
        return np.tile(att, (128, 1)).astype(f16)

    af1_h, af2_h = amat(inputs["att1"]), amat(inputs["att2"])
    b1full = np.tile(np.asarray(inputs["bias1"], np.float32)[None, :], (128, 1))
    b2full = np.tile(np.asarray(inputs["bias2"], np.float32)[None, :], (128, 1))
    ecols = np.zeros((128, NT * GPC), f16)
    for t in range(NT):
        ecols[:, t * GPC + t] = 1.0
    ident = np.eye(128, dtype=f16)
    wa_h = _interleave_k(_aug_w(np.asarray(inputs["Wa"], np.float32),
                                np.asarray(inputs["ba"], np.float32), 640, 2560),
                         5).astype(f16).reshape(128, 5 * 2560)
    wb_h = _interleave_k(_aug_w(np.asarray(inputs["Wb"], np.float32),
                                np.asarray(inputs["bb"], np.float32), 17 * 128,
                                1024), 17).astype(f16).reshape(128, 17 * 1024)
    wf_h = _interleave_k(_aug_w(np.asarray(inputs["Wf"], np.float32),
                                np.asarray(inputs["bf"], np.float32), 640, 640),
                         5).astype(f16).reshape(128, 5 * 640)
    sabf = np.tile(np.array([[float(inputs["sa"]), float(inputs["sb"]),
                              float(inputs["sf"])]], np.float32), (16, 1))

    in_maps = []
    for k in range(NCORES):
        sl = slice(k * SHARD, (k + 1) * SHARD)
        xs = x[sl, 1:]                       # [2048, 512]
        xT = np.zeros((640, SHARD), np.float32)
        xT[:FT] = xs.T
        xT[FT] = 1.0
        xTa_h = _interleave_k(xT, 5).astype(f16).reshape(128, 5 * SHARD)
        tsl = slice(k * NT, (k + 1) * NT)
        in_maps.append({
            "xTa": xTa_h, "w1l": w1l_h, "w1r": w1r_h, "w2l": w2l_h,
            "w2r": w2r_h, "b2rows": b2rows, "attf1": af1_h, "attf2": af2_h,
            "b1full": b1full, "b2full": b2full,
            "sdt": sdt[tsl].astype(f16),
            "sj": np.ascontiguousarray(
                sjt[tsl].reshape(NT, NJ, 128, 128).transpose(0, 2, 1, 3)
            ).reshape(NT, 128, NJ * 128).astype(exd_np),
            "idx": idx[tsl],
            "ecols": ecols, "ident": ident,
            "wa": wa_h, "wb": wb_h, "wf": wf_h, "sabf": sabf,
        })

    return in_maps, LP, NJ, njt


_last_exec_ns = None


def kernel(**inputs):
    global _last_exec_ns
    in_maps, LP, NJ, njt = _make_inmaps(inputs)
    key = (LP, EX_DTYPE, tuple(njt))
    if key not in _cache:
        _cache[key] = _build(LP, NJ, EX_DTYPE, njt)
    nc = _cache[key]
    from concourse.bass_utils import run_bass_kernel_spmd
    res = run_bass_kernel_spmd(nc, in_maps, list(range(NCORES)))
    _last_exec_ns = res.exec_time_ns
    kernel._last_res = res
    z = np.concatenate([np.asarray(r["zout"]) for r in res.results], 0)
    gm = np.concatenate([np.asarray(r["gmout"]) for r in res.results], 0)
    return z.astype(np.float32), gm.astype(np.float32)


# revision 47
# speedup vs baseline: 1.0889x; 1.0889x over previous
"""Trainium2 Bass kernel for nn_LorentzGNN (2x GATv2 + Lorentz head), 8-core SPMD.

Sharding: nodes (and their in-edges) are partitioned contiguously across 8 cores
(2048 nodes each). Each core computes its shard's GAT transforms; the xl source
table is replicated via chunked AllGathers (Shared addr space) overlapped with
the transform matmuls. Per-edge work uses a single edge-major dma_gather plus
host-built 0/1 segment matrices fed to the PE as matmuls: xr[dst] broadcast is
a segment matmul, leaky-relu runs on the scalar engine (AF.Lrelu), attention
logits are fused multiply+reduce ops on the vector engine, and softmax
denominator + weighted aggregation are segment matmuls. Graph-level ops
(centroid accumulation, Lorentz MLP over the 16 graphs each core owns) are
fused into the layer-2 epilogue; host concatenates per-core outputs.
"""
import numpy as np
import ml_dtypes

# ---------- problem constants (hardcoded per contract) ----------
N, E, B = 16384, 131072, 128
FT, HEADS, C = 512, 4, 128
NCORES = 8
SHARD = N // NCORES            # 2048
P = 128
NT = SHARD // P                # 16 dst tiles per core
GPC = B // NCORES              # 16 graphs per core
LEAK = 0.2
NCHUNK = 4                     # AllGather chunks per layer

_cache = {}


# ============================ host-side prep ============================

def _prep_edges(edge_index):
    src = np.concatenate([edge_index[0], np.arange(N)]).astype(np.int64)
    dst = np.concatenate([edge_index[1], np.arange(N)]).astype(np.int64)
    # remap src ids to the chunk-major AllGathered table layout:
    # node n = k*SHARD + g*CR + r lives at table row g*(8*CR) + k*CR + r
    CR = SHARD // NCHUNK
    src = (src % SHARD // CR) * (NCORES * CR) + (src // SHARD) * CR + src % CR
    order = np.argsort(dst, kind="stable")
    src, dst = src[order], dst[order]
    ntiles = N // P
    counts = np.bincount(dst // P, minlength=ntiles)
    LP = int(np.ceil(max(counts.max(), 1) / 128) * 128)
    NJ = LP // P
    srcs = np.zeros((ntiles, LP), np.int16)
    dloc = np.full((ntiles, LP), -1, np.int32)
    starts = np.concatenate([[0], np.cumsum(counts)])
    # per tile-slot block count: max over the 8 cores sharing that slot
    nt = ntiles // NCORES
    cmax = counts.reshape(NCORES, nt).max(0)
    njt = [int(np.ceil(max(c, 1) / 128)) for c in cmax]
    for t in range(ntiles):
        c = counts[t]
        srcs[t, :c] = src[starts[t]:starts[t] + c]
        dloc[t, :c] = dst[starts[t]:starts[t] + c] - t * P
    # segment matrices
    sdt = np.zeros((ntiles, P, LP), np.float32)          # [dst, j]
    jj = np.arange(LP)
    for t in range(ntiles):
        v = dloc[t] >= 0
        sdt[t, dloc[t, v], jj[v]] = 1.0
    sjt = np.ascontiguousarray(sdt.transpose(0, 2, 1))   # [j, dst] edge-major
    # idx buffers wrapped in 16 partitions, replicated to 128
    idx = srcs.reshape(ntiles, LP // 16, 16).transpose(0, 2, 1)  # [t, 16, LP/16]
    idx = np.tile(idx, (1, 8, 1)).astype(np.int16)               # [t, 128, LP/16]
    return srcs, sdt, sjt, idx, LP, NJ, njt


def _interleave_k(w, kchunks):
    """[K*128, N] -> [128, K, N] -> host layout [128, K*N] for SBUF."""
    K, Nn = w.shape
    assert K == kchunks * 128
    return np.ascontiguousarray(w.reshape(kchunks, 128, Nn).transpose(1, 0, 2))


def _aug_w(W, b, kpad, npad=None):
    """stack rows [W; b; 0-pad] to kpad rows, optionally pad cols to npad."""
    K, Nn = W.shape
    out = np.zeros((kpad, Nn if npad is None else npad), np.float32)
    out[:K, :Nn] = W
    out[K, :Nn] = b
    return out


# ============================ kernel build ============================

def _build(LP, NJ, ex_dtype_name, njt=None):
    njt = list(njt) if njt is not None else [NJ] * NT
    import concourse.bass as bass
    import concourse.bacc as bacc
    import concourse.tile as tile
    from concourse import mybir
    from concourse.library_config import mlp as gpsimd_mlp

    f32, f16 = mybir.dt.float32, mybir.dt.float16
    bf16, i16 = mybir.dt.bfloat16, mybir.dt.int16
    EXD = {"float16": f16, "bfloat16": bf16}[ex_dtype_name]
    AF = mybir.ActivationFunctionType
    ALU = mybir.AluOpType
    SCT = [[(o, min(512, nj * 128 - o)) for o in range(0, nj * 128, 512)]
           for nj in njt]                 # per-slot gather slices
    NREG = sorted({n for sc in SCT for _, n in sc})
    CR = SHARD // NCHUNK      # rows per AllGather chunk (512)

    nc = bacc.Bacc("TRN2", target_bir_lowering=False, debug=False,
                   num_devices=NCORES)
    groups = [list(range(NCORES))]

    # ---- DRAM I/O (per-core, same program) ----
    xTa = nc.dram_tensor("xTa", [128, 5 * SHARD], f16, kind="ExternalInput")
    w1l = nc.dram_tensor("w1l", [128, 5 * FT], f16, kind="ExternalInput")
    w1r = nc.dram_tensor("w1r", [128, 5 * FT], f16, kind="ExternalInput")
    w2l = nc.dram_tensor("w2l", [128, 4 * FT], f16, kind="ExternalInput")
    w2r = nc.dram_tensor("w2r", [128, 4 * FT], f16, kind="ExternalInput")
    b2r_d = nc.dram_tensor("b2rows", [2, FT], f16, kind="ExternalInput")
    af1_d = nc.dram_tensor("attf1", [128, 2 * FT], f16, kind="ExternalInput")
    af2_d = nc.dram_tensor("attf2", [128, 2 * FT], f16, kind="ExternalInput")
    b1f_d = nc.dram_tensor("b1full", [128, FT], f32, kind="ExternalInput")
    b2f_d = nc.dram_tensor("b2full", [128, FT], f32, kind="ExternalInput")
    sdt_d = nc.dram_tensor("sdt", [NT, 128, LP], f16, kind="ExternalInput")
    sj_d = nc.dram_tensor("sj", [NT, 128, NJ * 128], EXD, kind="ExternalInput")
    idx_d = nc.dram_tensor("idx", [NT, 128, LP // 16], i16, kind="ExternalInput")
    ecols_d = nc.dram_tensor("ecols", [128, NT * GPC], f16, kind="ExternalInput")
    ident_d = nc.dram_tensor("ident", [128, 128], f16, kind="ExternalInput")
    wa_d = nc.dram_tensor("wa", [128, 5 * 2560], f16, kind="ExternalInput")
    wb_d = nc.dram_tensor("wb", [128, 17 * 1024], f16, kind="ExternalInput")
    wf_d = nc.dram_tensor("wf", [128, 5 * 640], f16, kind="ExternalInput")
    sabf_d = nc.dram_tensor("sabf", [16, 3], f32, kind="ExternalInput")  # sa,sb,sf

    xl1_sh = nc.dram_tensor("xl1_sh", [SHARD, FT], f16)
    xl2_sh = nc.dram_tensor("xl2_sh", [SHARD, FT], f16)
    xl1_tb = nc.dram_tensor("xl1_tb", [N, FT], f16, addr_space="Shared")
    xl2_tb = nc.dram_tensor("xl2_tb", [N, FT], f16, addr_space="Shared")
    zout = nc.dram_tensor("zout", [GPC, FT + 1], f32, kind="ExternalOutput")
    gmout = nc.dram_tensor("gmout", [GPC, FT + 1], f32, kind="ExternalOutput")

    with tile.TileContext(nc, num_cores=NCORES) as tc:
        import contextlib
        est = contextlib.ExitStack()
        with est:
            nc.gpsimd.load_library(gpsimd_mlp)
            nregs = {n: nc.gpsimd.to_reg(n) for n in NREG}
            cpool = est.enter_context(tc.tile_pool(name="consts", bufs=1))
            wpool = est.enter_context(tc.tile_pool(name="wmlp", bufs=1))
            xrp = est.enter_context(tc.tile_pool(name="xr", bufs=1))
            h1p = est.enter_context(tc.tile_pool(name="h1", bufs=1))
            sbp = est.enter_context(tc.tile_pool(name="stream", bufs=2))
            smp = est.enter_context(tc.tile_pool(name="small", bufs=2))
            msb = est.enter_context(tc.tile_pool(name="mstream", bufs=2))
            psb = est.enter_context(tc.tile_pool(name="psb", bufs=4, space="PSUM"))
            pss = est.enter_context(tc.tile_pool(name="pss", bufs=2, space="PSUM"))
            pgmp = est.enter_context(tc.tile_pool(name="pgm", bufs=1, space="PSUM"))

            # ---- consts ----
            w1l_s = cpool.tile([128, 5 * FT], f16, name="w1l_s")
            nc.sync.dma_start(w1l_s[:], w1l[:])
            w1r_s = cpool.tile([128, 5 * FT], f16, name="w1r_s")
            nc.sync.dma_start(w1r_s[:], w1r[:])
            w2l_s = cpool.tile([128, 4 * FT], f16, name="w2l_s")
            nc.sync.dma_start(w2l_s[:], w2l[:])
            w2r_s = cpool.tile([128, 4 * FT], f16, name="w2r_s")
            nc.sync.dma_start(w2r_s[:], w2r[:])
            b2la_s = cpool.tile([1, FT], f16, name="b2la_s")
            nc.sync.dma_start(b2la_s[:], b2r_d[0:1, :])
            b2ra_s = cpool.tile([1, FT], f16, name="b2ra_s")
            nc.sync.dma_start(b2ra_s[:], b2r_d[1:2, :])
            af1_s = cpool.tile([128, 2 * FT], f16, name="af1_s")
            nc.sync.dma_start(af1_s[:], af1_d[:])
            af2_s = cpool.tile([128, 2 * FT], f16, name="af2_s")
            nc.sync.dma_start(af2_s[:], af2_d[:])
            b1f_s = cpool.tile([128, FT], f32, name="b1f_s")
            nc.sync.dma_start(b1f_s[:], b1f_d[:])
            b2f_s = cpool.tile([128, FT], f32, name="b2f_s")
            nc.sync.dma_start(b2f_s[:], b2f_d[:])
            ident_s = cpool.tile([128, 128], f16, name="ident_s")
            nc.sync.dma_start(ident_s[:], ident_d[:])
            ecols_s = cpool.tile([128, NT * GPC], f16, name="ecols_s")
            nc.sync.dma_start(ecols_s[:], ecols_d[:])
            ones1 = cpool.tile([1, FT], f16, name="ones1")
            nc.vector.memset(ones1[:], 1.0)
            sabf_s = cpool.tile([16, 3], f32, name="sabf_s")
            nc.sync.dma_start(sabf_s[:], sabf_d[:])
            esc = cpool.tile([16, 3], f32, name="esc")
            nc.scalar.activation(esc[:], sabf_s[:], AF.Exp)
            z0p = cpool.tile([16, 640], f16, name="z0p")
            nc.vector.memset(z0p[:], 0.0)
            nc.vector.memset(z0p[:, 513:514], 1.0)
            onescol = cpool.tile([128, 1], f16, name="onescol")
            nc.vector.memset(onescol[:], 1.0)
            tacc = cpool.tile([128, NT], f32, name="tacc")
            alph = cpool.tile([128, 1], f32, name="alph")
            nc.vector.memset(alph[:], LEAK)
            nege = cpool.tile([128, 1], f32, name="nege")
            nc.vector.memset(nege[:], -2.0)
            # MLP weights rotate through one 35KB buffer: wa early (overlaps
            # the GNN), wb/wf reload behind each llin stage.
            wa_s = wpool.tile([128, 17 * 1024], f16, tag="w", name="wa_s", bufs=1)
            nc.sync.dma_start(wa_s[:, 0:5 * 2560], wa_d[:])
            wa_v = wa_s[:, 0:5 * 2560].rearrange("p (k n) -> p k n", k=5)

            def nsqrt(out_ap, x_ap, pool, pfx):
                """out = sqrt(x), Newton-refined (ACT sqrt LUT is ~4e-3)."""
                y0 = pool.tile(list(x_ap.shape), f32, tag="nsq", name=pfx + "y0",
                               bufs=6)
                nc.scalar.activation(y0[:], x_ap, AF.Sqrt)
                r0 = pool.tile(list(x_ap.shape), f32, tag="nsq", name=pfx + "r0",
                               bufs=6)
                nc.vector.reciprocal(r0[:], y0[:])
                nc.vector.tensor_tensor(out=r0[:], in0=x_ap, in1=r0[:],
                                        op=ALU.mult)
                nc.vector.tensor_tensor(out=y0[:], in0=y0[:], in1=r0[:],
                                        op=ALU.add)
                nc.vector.tensor_scalar_mul(out_ap, y0[:], 0.5)

            xr_s = xrp.tile([128, NT * FT], f16, name="xr_s")       # resident xr
            h1_s = h1p.tile([128, NT * FT], f16, name="h1_s")       # resident h1
            h1pre = h1p.tile([128, NT * FT], f16, name="h1pre")     # pre-gelu

            def allgather(g, sh, tb):
                # table rows are chunk-major (g, k, r): each chunk's gathered
                # output is one contiguous [8*CR, FT] slice (BIR requires it)
                nc.gpsimd.collective_compute(
                    "AllGather", ALU.bypass, replica_groups=groups,
                    ins=[sh[g * CR:(g + 1) * CR, :]],
                    outs=[tb[g * NCORES * CR:(g + 1) * NCORES * CR, :]])

            def transform1():
                xTa_v = xTa[:].rearrange("p (k n) -> p k n", k=5)
                w1l_v = w1l_s[:].rearrange("p (k n) -> p k n", k=5)
                w1r_v = w1r_s[:].rearrange("p (k n) -> p k n", k=5)
                for t in range(NT):
                    xt = smp.tile([128, 5 * 128], f16, tag="xTa_t", name="xt",
                                  bufs=3)
                    nc.sync.dma_start(
                        xt[:].rearrange("p (k n) -> p k n", k=5),
                        xTa_v[:, :, t * 128:(t + 1) * 128])
                    xt_v = xt[:].rearrange("p (k n) -> p k n", k=5)
                    pl = psb.tile([128, FT], f32, tag="pbig", name="pl")
                    pr = psb.tile([128, FT], f32, tag="pbig", name="pr")
                    for kc in range(5):
                        nc.tensor.matmul(pl[:], lhsT=xt_v[:, kc, :],
                                         rhs=w1l_v[:, kc, :],
                                         start=(kc == 0), stop=(kc == 4))
                        nc.tensor.matmul(pr[:], lhsT=xt_v[:, kc, :],
                                         rhs=w1r_v[:, kc, :],
                                         start=(kc == 0), stop=(kc == 4))
                    xlt = smp.tile([128, FT], f16, tag="xlt", name="xlt")
                    nc.scalar.activation(xlt[:], pl[:], AF.Copy)
                    nc.scalar.activation(xr_s[:, t * FT:(t + 1) * FT], pr[:],
                                         AF.Copy)
                    nc.sync.dma_start(xl1_sh[t * 128:(t + 1) * 128, :], xlt[:])
                    if (t + 1) % (NT // NCHUNK) == 0:
                        allgather(t // (NT // NCHUNK), xl1_sh, xl1_tb)

            def transform2():
                w2l_v = w2l_s[:].rearrange("p (k n) -> p k n", k=4)
                w2r_v = w2r_s[:].rearrange("p (k n) -> p k n", k=4)
                for t in range(NT):
                    h1t = h1_s[:].rearrange("p (t n) -> p t n", t=NT)[:, t, :]
                    h1T = smp.tile([128, 4 * 128], f16, tag="h1T", name="h1T")
                    for fc in range(4):
                        pt = pss.tile([128, 128], f16, tag="pe", name="pt",
                                      bufs=1)
                        nc.tensor.transpose(pt[:], h1t[:, fc * 128:(fc + 1) * 128],
                                            ident_s[:])
                        nc.scalar.activation(h1T[:, fc * 128:(fc + 1) * 128],
                                             pt[:], AF.Copy)
                    pl = psb.tile([128, FT], f32, tag="pbig", name="pl2")
                    pr = psb.tile([128, FT], f32, tag="pbig", name="pr2")
                    h1T_v = h1T[:].rearrange("p (k n) -> p k n", k=4)
                    for kc in range(4):
                        nc.tensor.matmul(pl[:], lhsT=h1T_v[:, kc, :],
                                         rhs=w2l_v[:, kc, :],
                                         start=(kc == 0), stop=False)
                        nc.tensor.matmul(pr[:], lhsT=h1T_v[:, kc, :],
                                         rhs=w2r_v[:, kc, :],
                                         start=(kc == 0), stop=False)
                    nc.tensor.matmul(pl[:], lhsT=ones1[:, 0:128], rhs=b2la_s[:],
                                     start=False, stop=True)
                    nc.tensor.matmul(pr[:], lhsT=ones1[:, 0:128], rhs=b2ra_s[:],
                                     start=False, stop=True)
                    xlt = smp.tile([128, FT], f16, tag="xlt", name="xlt2")
                    nc.scalar.activation(xlt[:], pl[:], AF.Copy)
                    nc.scalar.activation(xr_s[:, t * FT:(t + 1) * FT], pr[:],
                                         AF.Copy)
                    nc.sync.dma_start(xl2_sh[t * 128:(t + 1) * 128, :], xlt[:])
                    if (t + 1) % (NT // NCHUNK) == 0:
                        allgather(t // (NT // NCHUNK), xl2_sh, xl2_tb)

            def edge_layer(layer, table, af_s):
                """GATv2 message passing; writes h1_s (layer1) or, for layer2,
                the h2 epilogue + centroid accumulation + z0 extraction."""
                if layer == 2:
                    pgm = pgmp.tile([128, 640], f32, name="pgm")
                for t in range(NT):
                    NJt = njt[t]
                    idxt = smp.tile([128, LP // 16], i16, tag="idxt",
                                    name="idxt", bufs=4)
                    nc.sync.dma_start(idxt[:, 0:NJt * 8],
                                      idx_d[t, :, 0:NJt * 8])
                    sdtt = sbp.tile([128, LP], f16, tag="sdtt", name="sdtt")
                    nc.sync.dma_start(sdtt[:, 0:NJt * 128],
                                      sdt_d[t, :, 0:NJt * 128])
                    sjt = sbp.tile([128, NJ * 128], EXD, tag="sjt", name="sjt")
                    nc.sync.dma_start(sjt[:, 0:NJt * 128],
                                      sj_d[t, :, 0:NJt * 128])
                    sj_v = sjt[:].rearrange("p (j d) -> p j d", j=NJ)

                    xlg = sbp.tile([128, NJ * FT], f16, tag="xlg", name="xlg")
                    xlg_w = xlg[:].rearrange("p (j n) -> p j n", j=NJ)
                    for (o, n) in SCT[t]:
                        nc.gpsimd.dma_gather(
                            xlg_w[:, o // 128:(o + n) // 128, :], table[:],
                            idxt[:, o // 16:(o + n) // 16], n, nregs[n], FT)
                    xlg_v = xlg[:].rearrange("p (j n) -> p j n", j=NJ)

                    xr_t = xr_s[:].rearrange("p (t n) -> p t n", t=NT)[:, t, :]
                    logit = smp.tile([128, NJ * HEADS], f32, tag="logit",
                                     name="logit")
                    PAIRS = [(jb, min(2, NJt - jb)) for jb in range(0, NJt, 2)]
                    for (jb, w) in PAIRS:
                        lr2 = smp.tile([128, 2 * FT], f16, tag="lr", name="lr",
                                       bufs=2)
                        for sub in range(w):
                            j = jb + sub
                            ps2 = psb.tile([128, FT], f32, tag="pbig",
                                           name="ps2")
                            nc.tensor.matmul(ps2[:],
                                             lhsT=sdtt[:, j * 128:(j + 1) * 128],
                                             rhs=xr_t, start=True, stop=False)
                            nc.tensor.matmul(ps2[:], lhsT=ident_s[:],
                                             rhs=xlg_v[:, j, :],
                                             start=False, stop=True)
                            nc.scalar.activation(
                                lr2[:, sub * FT:(sub + 1) * FT], ps2[:],
                                AF.Prelu, alpha=alph[:])
                        tp2 = smp.tile([128, 2 * FT], f16, tag="tp2",
                                       name="tp2", bufs=1)
                        nc.vector.tensor_tensor(
                            out=tp2[:, 0:w * FT], in0=lr2[:, 0:w * FT],
                            in1=af_s[:, 0:w * FT], op=ALU.mult)
                        nc.vector.tensor_reduce(
                            logit[:, jb * HEADS:(jb + w) * HEADS],
                            tp2[:, 0:w * FT].rearrange("p (g c) -> p g c",
                                                       g=w * HEADS),
                            axis=mybir.AxisListType.X, op=ALU.add)
                    exf = smp.tile([128, NJ * HEADS], f32, tag="exf", name="exf")
                    nc.scalar.activation(exf[:, 0:NJt * HEADS],
                                         logit[:, 0:NJt * HEADS], AF.Exp,
                                         bias=nege[:])
                    ex = smp.tile([128, NJ * HEADS], EXD, tag="ex", name="ex")
                    nc.scalar.activation(ex[:, 0:NJt * HEADS],
                                         exf[:, 0:NJt * HEADS], AF.Copy)
                    ex_v = ex[:].rearrange("p (j h) -> p j h", j=NJ)
                    pden = pss.tile([128, HEADS], f32, tag="pden", name="pden",
                                    bufs=1)
                    pagg = psb.tile([128, FT], f32, tag="pbig", name="pagg")
                    for (jb, w) in PAIRS:
                        wt2 = smp.tile([128, 2 * FT], EXD, tag="wt", name="wt",
                                       bufs=2)
                        nc.vector.tensor_tensor(
                            out=wt2[:, 0:w * FT].rearrange(
                                "p (j h c) -> p j h c", j=w, h=HEADS),
                            in0=xlg_v[:, jb:jb + w, :].rearrange(
                                "p j (h c) -> p j h c", h=HEADS),
                            in1=ex_v[:, jb:jb + w, :].broadcast_to(
                                [128, w, HEADS, C]),
                            op=ALU.mult)
                        for sub in range(w):
                            j = jb + sub
                            nc.tensor.matmul(pden[:], lhsT=sj_v[:, j, :],
                                             rhs=ex_v[:, j, :],
                                             start=(j == 0),
                                             stop=(j == NJt - 1))
                            nc.tensor.matmul(pagg[:], lhsT=sj_v[:, j, :],
                                             rhs=wt2[:, sub * FT:
                                                     (sub + 1) * FT],
                                             start=(j == 0),
                                             stop=(j == NJt - 1))
                    rden = smp.tile([128, HEADS], f32, tag="rden", name="rden")
                    nc.vector.reciprocal(rden[:], pden[:])
                    # epilogue: out = pagg*rden (per head) + bias (on gpsimd)
                    if layer == 1:
                        for h in range(HEADS):
                            nc.vector.scalar_tensor_tensor(
                                out=h1pre[:, t * FT + h * C:t * FT + (h + 1) * C],
                                in0=pagg[:, h * C:(h + 1) * C],
                                scalar=rden[:, h:h + 1],
                                in1=b1f_s[:, h * C:(h + 1) * C],
                                op0=ALU.mult, op1=ALU.add)
                        if (t + 1) % (NT // NCHUNK) == 0:
                            g = t // (NT // NCHUNK)
                            for tg in range(g * (NT // NCHUNK), (g + 1) *
                                            (NT // NCHUNK)):
                                nc.scalar.activation(
                                    h1_s[:, tg * FT:(tg + 1) * FT],
                                    h1pre[:, tg * FT:(tg + 1) * FT], AF.Gelu)
                    else:
                        h2sp = smp.tile([128, FT], f32, tag="h2sp", name="h2sp")
                        for h in range(HEADS):
                            nc.vector.scalar_tensor_tensor(
                                out=h2sp[:, h * C:(h + 1) * C],
                                in0=pagg[:, h * C:(h + 1) * C],
                                scalar=rden[:, h:h + 1],
                                in1=b2f_s[:, h * C:(h + 1) * C],
                                op0=ALU.mult, op1=ALU.add)
                        sqj = smp.tile([128, FT], f16, tag="sqj", name="sqj")
                        nc.vector.scalar_tensor_tensor(
                            out=sqj[:], in0=h2sp[:], scalar=1.0, in1=h2sp[:],
                            op0=ALU.mult, op1=ALU.mult,
                            accum_out=tacc[:, t:t + 1])
                        h2c = smp.tile([128, FT], f16, tag="h2c", name="h2c")
                        nc.scalar.activation(h2c[:], h2sp[:], AF.Copy)
                        ec = ecols_s[:, t * GPC:(t + 1) * GPC]
                        nc.tensor.matmul(pgm[:GPC, 0:FT], lhsT=ec, rhs=h2c[:],
                                         start=(t == 0), stop=(t == NT - 1))
                        nc.sync.dma_start(z0p[t:t + 1, 1:FT + 1], h2c[0:1, :])
                if layer == 2:
                    # batched time coordinate: t = sqrt(1 + |s|^2) for all tiles
                    nc.vector.tensor_scalar_add(tacc[:], tacc[:], 1.0)
                    tsq = smp.tile([128, NT], f32, tag="tsq", name="tsq")
                    nsqrt(tsq[:], tacc[:], smp, "t_")
                    tc16 = smp.tile([128, NT], f16, tag="tc16", name="tc16")
                    nc.vector.tensor_copy(tc16[:], tsq[:])
                    nc.tensor.matmul(pgm[:GPC, FT:FT + 1], lhsT=tc16[:],
                                     rhs=onescol[:], start=True, stop=True)
                    # z0 time coord recomputed from its (f16) space part
                    zsqj = msb.tile([16, FT], f16, tag="zsqj", name="zsqj")
                    zta = msb.tile([16, 1], f32, tag="t1", name="zta", bufs=8)
                    nc.vector.scalar_tensor_tensor(
                        out=zsqj[:], in0=z0p[:, 1:FT + 1], scalar=1.0,
                        in1=z0p[:, 1:FT + 1], op0=ALU.mult, op1=ALU.mult,
                        accum_out=zta[:])
                    nc.vector.tensor_scalar_add(zta[:], zta[:], 1.0)
                    ztb = msb.tile([16, 1], f32, tag="t1", name="ztb", bufs=8)
                    nsqrt(ztb[:], zta[:], msb, "zt_")
                    nc.scalar.activation(z0p[:, 0:1], ztb[:], AF.Copy)
                    return pgm

            transform1()
            edge_layer(1, xl1_tb, af1_s)
            transform2()
            pgm = edge_layer(2, xl2_tb, af2_s)

            # -------- centroid epilogue (pgm layout: [space(512) | time]) ----
            sums = smp.tile([GPC, FT + 1], f32, tag="sums", name="sums")
            nc.scalar.activation(sums[:], pgm[:GPC, 0:FT + 1], AF.Copy)
            sqgj = smp.tile([GPC, FT], f16, tag="sqgj", name="sqgj")
            sa_ = smp.tile([GPC, 1], f32, tag="sacc", name="sa_")
            nc.vector.scalar_tensor_tensor(
                out=sqgj[:], in0=sums[:, 0:FT], scalar=1.0, in1=sums[:, 0:FT],
                op0=ALU.mult, op1=ALU.mult, accum_out=sa_[:])
            innr = smp.tile([GPC, 1], f32, tag="in1", name="innr")
            nc.vector.tensor_tensor(out=innr[:], in0=sums[:, FT:FT + 1],
                                    in1=sums[:, FT:FT + 1], op=ALU.mult)
            nc.vector.tensor_tensor(out=innr[:], in0=innr[:], in1=sa_[:],
                                    op=ALU.subtract)
            nc.vector.tensor_scalar_max(innr[:], innr[:], 1e-8 * (N // B) ** 2)
            rt = smp.tile([GPC, 1], f32, tag="in1", name="rt")
            nsqrt(rt[:], innr[:], smp, "g_")
            nc.vector.reciprocal(rt[:], rt[:])
            gmt = smp.tile([GPC, FT + 1], f32, tag="sums", name="gmt")
            nc.scalar.activation(gmt[:, 0:1], sums[:, FT:FT + 1], AF.Copy,
                                 scale=rt[:])
            nc.scalar.activation(gmt[:, 1:FT + 1], sums[:, 0:FT], AF.Copy,
                                 scale=rt[:])
            nc.sync.dma_start(gmout[:], gmt[:])

            # ---------------- Lorentz MLP on z0 [16, 513] ----------------
            def trans_blocks(zp, kb):
                """zp [16, kb*128] f16 -> zT [128, kb*16] f16 via PE."""
                zT = msb.tile([128, 17 * 16], f16, tag="zT", name="zT")
                for k in range(kb):
                    pt = pss.tile([128, 128], f16, tag="pe", name="ptm",
                                  bufs=1)
                    nc.tensor.transpose(pt[:, 0:16], zp[:, k * 128:(k + 1) * 128],
                                        ident_s[:16, :16])
                    nc.scalar.activation(zT[:, k * 16:(k + 1) * 16], pt[:, 0:16],
                                         AF.Copy)
                return zT

            def mm_thin(zT, kb, w_v, ncols):
                """out [16, ncols] f32 = zT.T @ w; w_v view [128, kb, ncols]."""
                out = msb.tile([16, 2560], f32, tag="mlpo", name="out", bufs=1)
                zT_v = zT[:].rearrange("p (k n) -> p k n", k=17)
                for o in range(0, ncols, 512):
                    n = min(512, ncols - o)
                    pm = psb.tile([128, FT], f32, tag="pbig", name="pm")
                    for k in range(kb):
                        nc.tensor.matmul(pm[:16, :n], lhsT=zT_v[:, k, :16],
                                         rhs=w_v[:, k, o:o + n],
                                         start=(k == 0), stop=(k == kb - 1))
                    nc.scalar.activation(out[:, o:o + n], pm[:16, :n], AF.Copy)
                return out

            def llin_post(zz, kout, esc_idx):
                """returns (t1, r_) for zz [16, ncols>=kout] f32."""
                t1 = msb.tile([16, 1], f32, tag="t1", name="t1", bufs=8)
                nc.scalar.activation(t1[:], zz[:, 0:1], AF.Sigmoid)
                nc.vector.tensor_scalar(
                    out=t1[:], in0=t1[:],
                    scalar1=esc[:, esc_idx:esc_idx + 1],
                    scalar2=1.1, op0=ALU.mult, op1=ALU.add)
                sq = msb.tile([16, 2048], f32, tag="msq", name="sq", bufs=1)
                ac = msb.tile([16, 1], f32, tag="t1", name="ac", bufs=8)
                nc.scalar.activation(sq[:, :kout - 1], zz[:, 1:kout], AF.Square,
                                     accum_out=ac[:])
                nc.vector.tensor_scalar_max(ac[:], ac[:], 1e-8)
                r_ = msb.tile([16, 1], f32, tag="t1", name="r_", bufs=8)
                nc.vector.reciprocal(r_[:], ac[:])
                t2 = msb.tile([16, 1], f32, tag="t1", name="t2", bufs=8)
                nc.vector.tensor_tensor(out=t2[:], in0=t1[:], in1=t1[:],
                                        op=ALU.mult)
                nc.vector.tensor_scalar_add(t2[:], t2[:], -1.0)
                nc.vector.tensor_tensor(out=r_[:], in0=r_[:], in1=t2[:],
                                        op=ALU.mult)
                nsqrt(r_[:], r_[:], msb, "m_")
                return t1, r_

            # llin-a: z0p [16, 640] -> zA [16, 2560]
            zT = trans_blocks(z0p, 5)
            zA = mm_thin(zT, 5, wa_v, 2560)
            t1, r1 = llin_post(zA, 2049, 0)
            wb_s = wpool.tile([128, 17 * 1024], f16, tag="w", name="wb_s",
                              bufs=1)
            nc.sync.dma_start(wb_s[:], wb_d[:])
            wb_v = wb_s[:].rearrange("p (k n) -> p k n", k=17)
            # z1 = add_time(gelu(sp*r1)): gelu with scale=r1
            z1p = msb.tile([16, 17 * 128], f16, tag="z1p", name="z1p", bufs=1)
            nc.vector.memset(z1p[:], 0.0)
            nc.scalar.activation(z1p[:, 1:2049], zA[:, 1:2049], AF.Gelu,
                                 scale=r1[:])
            sqz = msb.tile([16, 2048], f32, tag="msq", name="sqz", bufs=1)
            az = msb.tile([16, 1], f32, tag="t1", name="az", bufs=8)
            nc.scalar.activation(sqz[:], z1p[:, 1:2049], AF.Square,
                                 accum_out=az[:])
            az1 = msb.tile([16, 1], f32, tag="t1", name="az1", bufs=8)
            nc.scalar.activation(az1[:], az[:], AF.Identity, bias=1.0)
            nsqrt(z1p[:, 0:1], az1[:], msb, "z_")
            nc.vector.memset(z1p[:, 2049:2050], 1.0)
            # llin-b: [16, 2049] -> [16, 513]
            zTb = trans_blocks(z1p, 17)
            zB = mm_thin(zTb, 17, wb_v, 1024)
            t3, r3 = llin_post(zB, 513, 1)
            wf_s = wpool.tile([128, 17 * 1024], f16, tag="w", name="wf_s",
                              bufs=1)
            nc.sync.dma_start(wf_s[:, 0:5 * 640], wf_d[:])
            wf_v = wf_s[:, 0:5 * 640].rearrange("p (k n) -> p k n", k=5)
            z2p = msb.tile([16, 640], f16, tag="z2p", name="z2p", bufs=1)
            nc.vector.memset(z2p[:], 0.0)
            nc.scalar.activation(z2p[:, 0:1], t3[:], AF.Copy)
            nc.scalar.activation(z2p[:, 1:513], zB[:, 1:513], AF.Copy,
                                 scale=r3[:])
            nc.vector.memset(z2p[:, 513:514], 1.0)
            # llin-f: [16, 513] -> [16, 513]
            zTf = trans_blocks(z2p, 5)
            zF = mm_thin(zTf, 5, wf_v, 640)
            t4, r4 = llin_post(zF, 513, 2)
            zfin = msb.tile([16, 640], f32, tag="zfin", name="zfin", bufs=1)
            nc.scalar.activation(zfin[:, 0:1], t4[:], AF.Copy)
            nc.scalar.activation(zfin[:, 1:513], zF[:, 1:513], AF.Copy,
                                 scale=r4[:])
            nc.sync.dma_start(zout[:], zfin[:, 0:FT + 1])

    nc.compile()
    return nc


# ============================ host entry ============================

EX_DTYPE = "float16"    # logits are small; exp shifted by -2


def _make_inmaps(inputs):
    x = np.asarray(inputs["x"], np.float32)
    edge_index = np.asarray(inputs["edge_index"])
    srcs, sdt, sjt, idx, LP, NJ, njt = _prep_edges(edge_index)

    f16 = np.float16
    exd_np = ml_dtypes.bfloat16 if EX_DTYPE == "bfloat16" else np.float16

    # ---- shared (replicated) host arrays ----
    def aug5(W, b):
        return _interleave_k(_aug_w(np.asarray(W, np.float32),
                                    np.asarray(b, np.float32), 640), 5)

    w1l_h = aug5(inputs["Wl1"], inputs["bl1"]).astype(f16).reshape(128, 5 * FT)
    w1r_h = aug5(inputs["Wr1"], inputs["br1"]).astype(f16).reshape(128, 5 * FT)
    w2l_h = _interleave_k(np.asarray(inputs["Wl2"], np.float32), 4
                          ).astype(f16).reshape(128, 4 * FT)
    w2r_h = _interleave_k(np.asarray(inputs["Wr2"], np.float32), 4
                          ).astype(f16).reshape(128, 4 * FT)
    b2rows = np.stack([np.asarray(inputs["bl2"]),
                       np.asarray(inputs["br2"])]).astype(f16)

    def amat(att):
        att = np.asarray(att, np.float32).reshape(1, HEADS * C)
        return np.tile(att, (128, 2)).astype(f16)

    af1_h, af2_h = amat(inputs["att1"]), amat(inputs["att2"])
    b1full = np.tile(np.asarray(inputs["bias1"], np.float32)[None, :], (128, 1))
    b2full = np.tile(np.asarray(inputs["bias2"], np.float32)[None, :], (128, 1))
    ecols = np.zeros((128, NT * GPC), f16)
    for t in range(NT):
        ecols[:, t * GPC + t] = 1.0
    ident = np.eye(128, dtype=f16)
    wa_h = _interleave_k(_aug_w(np.asarray(inputs["Wa"], np.float32),
                                np.asarray(inputs["ba"], np.float32), 640, 2560),
                         5).astype(f16).reshape(128, 5 * 2560)
    wb_h = _interleave_k(_aug_w(np.asarray(inputs["Wb"], np.float32),
                                np.asarray(inputs["bb"], np.float32), 17 * 128,
                                1024), 17).astype(f16).reshape(128, 17 * 1024)
    wf_h = _interleave_k(_aug_w(np.asarray(inputs["Wf"], np.float32),
                                np.asarray(inputs["bf"], np.float32), 640, 640),
                         5).astype(f16).reshape(128, 5 * 640)
    sabf = np.tile(np.array([[float(inputs["sa"]), float(inputs["sb"]),
                              float(inputs["sf"])]], np.float32), (16, 1))

    in_maps = []
    for k in range(NCORES):
        sl = slice(k * SHARD, (k + 1) * SHARD)
        xs = x[sl, 1:]                       # [2048, 512]
        xT = np.zeros((640, SHARD), np.float32)
        xT[:FT] = xs.T
        xT[FT] = 1.0
        xTa_h = _interleave_k(xT, 5).astype(f16).reshape(128, 5 * SHARD)
        tsl = slice(k * NT, (k + 1) * NT)
        in_maps.append({
            "xTa": xTa_h, "w1l": w1l_h, "w1r": w1r_h, "w2l": w2l_h,
            "w2r": w2r_h, "b2rows": b2rows, "attf1": af1_h, "attf2": af2_h,
            "b1full": b1full, "b2full": b2full,
            "sdt": sdt[tsl].astype(f16),
            "sj": np.ascontiguousarray(
                sjt[tsl].reshape(NT, NJ, 128, 128).transpose(0, 2, 1, 3)
            ).reshape(NT, 128, NJ * 128).astype(exd_np),
            "idx": idx[tsl],
            "ecols": ecols, "ident": ident,
            "wa": wa_h, "wb": wb_h, "wf": wf_h, "sabf": sabf,
        })

    return in_maps, LP, NJ, njt


_last_exec_ns = None


def kernel(**inputs):
    global _last_exec_ns
    in_maps, LP, NJ, njt = _make_inmaps(inputs)
    key = (LP, EX_DTYPE, tuple(njt))
    if key not in _cache:
        _cache[key] = _build(LP, NJ, EX_DTYPE, njt)
    nc = _cache[key]
    from concourse.bass_utils import run_bass_kernel_spmd
    res = run_bass_kernel_spmd(nc, in_maps, list(range(NCORES)))
    _last_exec_ns = res.exec_time_ns
    kernel._last_res = res
    z = np.concatenate([np.asarray(r["zout"]) for r in res.results], 0)
    gm = np.concatenate([np.asarray(r["gmout"]) for r in res.results], 0)
    return z.astype(np.float32), gm.astype(np.float32)


# revision 52
# speedup vs baseline: 1.1532x; 1.0590x over previous
"""Trainium2 Bass kernel for nn_LorentzGNN (2x GATv2 + Lorentz head), 8-core SPMD.

Sharding: nodes (and their in-edges) are partitioned contiguously across 8 cores
(2048 nodes each). Each core computes its shard's GAT transforms; the xl source
table is replicated via chunked AllGathers (Shared addr space) overlapped with
the transform matmuls. Per-edge work uses a single edge-major dma_gather plus
host-built 0/1 segment matrices fed to the PE as matmuls: xr[dst] broadcast is
a segment matmul, leaky-relu runs on the scalar engine (AF.Lrelu), attention
logits are fused multiply+reduce ops on the vector engine, and softmax
denominator + weighted aggregation are segment matmuls. Graph-level ops
(centroid accumulation, Lorentz MLP over the 16 graphs each core owns) are
fused into the layer-2 epilogue; host concatenates per-core outputs.
"""
import numpy as np
import ml_dtypes

# ---------- problem constants (hardcoded per contract) ----------
N, E, B = 16384, 131072, 128
FT, HEADS, C = 512, 4, 128
NCORES = 8
SHARD = N // NCORES            # 2048
P = 128
NT = SHARD // P                # 16 dst tiles per core
GPC = B // NCORES              # 16 graphs per core
LEAK = 0.2
NCHUNK = 4                     # AllGather chunks per layer

_cache = {}


# ============================ host-side prep ============================

def _prep_edges(edge_index):
    src = np.concatenate([edge_index[0], np.arange(N)]).astype(np.int64)
    dst = np.concatenate([edge_index[1], np.arange(N)]).astype(np.int64)
    # remap src ids to the chunk-major AllGathered table layout:
    # node n = k*SHARD + g*CR + r lives at table row g*(8*CR) + k*CR + r
    CR = SHARD // NCHUNK
    src = (src % SHARD // CR) * (NCORES * CR) + (src // SHARD) * CR + src % CR
    order = np.argsort(dst, kind="stable")
    src, dst = src[order], dst[order]
    ntiles = N // P
    counts = np.bincount(dst // P, minlength=ntiles)
    LP = int(np.ceil(max(counts.max(), 1) / 128) * 128)
    NJ = LP // P
    srcs = np.zeros((ntiles, LP), np.int16)
    dloc = np.full((ntiles, LP), -1, np.int32)
    starts = np.concatenate([[0], np.cumsum(counts)])
    # per tile-slot block count: max over the 8 cores sharing that slot
    nt = ntiles // NCORES
    cmax = counts.reshape(NCORES, nt).max(0)
    njt = [int(np.ceil(max(c, 1) / 128)) for c in cmax]
    for t in range(ntiles):
        c = counts[t]
        srcs[t, :c] = src[starts[t]:starts[t] + c]
        dloc[t, :c] = dst[starts[t]:starts[t] + c] - t * P
    # segment matrices
    sdt = np.zeros((ntiles, P, LP), np.float32)          # [dst, j]
    jj = np.arange(LP)
    for t in range(ntiles):
        v = dloc[t] >= 0
        sdt[t, dloc[t, v], jj[v]] = 1.0
    sjt = np.ascontiguousarray(sdt.transpose(0, 2, 1))   # [j, dst] edge-major
    # idx buffers wrapped in 16 partitions, replicated to 128
    idx = srcs.reshape(ntiles, LP // 16, 16).transpose(0, 2, 1)  # [t, 16, LP/16]
    idx = np.tile(idx, (1, 8, 1)).astype(np.int16)               # [t, 128, LP/16]
    return srcs, sdt, sjt, idx, LP, NJ, njt


def _interleave_k(w, kchunks):
    """[K*128, N] -> [128, K, N] -> host layout [128, K*N] for SBUF."""
    K, Nn = w.shape
    assert K == kchunks * 128
    return np.ascontiguousarray(w.reshape(kchunks, 128, Nn).transpose(1, 0, 2))


def _aug_w(W, b, kpad, npad=None):
    """stack rows [W; b; 0-pad] to kpad rows, optionally pad cols to npad."""
    K, Nn = W.shape
    out = np.zeros((kpad, Nn if npad is None else npad), np.float32)
    out[:K, :Nn] = W
    out[K, :Nn] = b
    return out


# ============================ kernel build ============================

def _build(LP, NJ, ex_dtype_name, njt=None):
    njt = list(njt) if njt is not None else [NJ] * NT
    import concourse.bass as bass
    import concourse.bacc as bacc
    import concourse.tile as tile
    from concourse import mybir
    from concourse.library_config import mlp as gpsimd_mlp

    f32, f16 = mybir.dt.float32, mybir.dt.float16
    bf16, i16 = mybir.dt.bfloat16, mybir.dt.int16
    EXD = {"float16": f16, "bfloat16": bf16}[ex_dtype_name]
    AF = mybir.ActivationFunctionType
    ALU = mybir.AluOpType
    SCT = [[(o, min(512, nj * 128 - o)) for o in range(0, nj * 128, 512)]
           for nj in njt]                 # per-slot gather slices
    NREG = sorted({n for sc in SCT for _, n in sc})
    CR = SHARD // NCHUNK      # rows per AllGather chunk (512)

    nc = bacc.Bacc("TRN2", target_bir_lowering=False, debug=False,
                   num_devices=NCORES)
    groups = [list(range(NCORES))]

    # ---- DRAM I/O (per-core, same program) ----
    xTa = nc.dram_tensor("xTa", [128, 5 * SHARD], f16, kind="ExternalInput")
    w1l = nc.dram_tensor("w1l", [128, 5 * FT], f16, kind="ExternalInput")
    w1r = nc.dram_tensor("w1r", [128, 5 * FT], f16, kind="ExternalInput")
    w2l = nc.dram_tensor("w2l", [128, 4 * FT], f16, kind="ExternalInput")
    w2r = nc.dram_tensor("w2r", [128, 4 * FT], f16, kind="ExternalInput")
    b2r_d = nc.dram_tensor("b2rows", [2, FT], f16, kind="ExternalInput")
    af1_d = nc.dram_tensor("attf1", [128, 2 * FT], f16, kind="ExternalInput")
    af2_d = nc.dram_tensor("attf2", [128, 2 * FT], f16, kind="ExternalInput")
    b1f_d = nc.dram_tensor("b1full", [128, FT], f32, kind="ExternalInput")
    b2f_d = nc.dram_tensor("b2full", [128, FT], f32, kind="ExternalInput")
    sdt_d = nc.dram_tensor("sdt", [NT, 128, LP], f16, kind="ExternalInput")
    sj_d = nc.dram_tensor("sj", [NT, 128, NJ * 128], EXD, kind="ExternalInput")
    idx_d = nc.dram_tensor("idx", [NT, 128, LP // 16], i16, kind="ExternalInput")
    ecols_d = nc.dram_tensor("ecols", [128, NT * GPC], f16, kind="ExternalInput")
    ident_d = nc.dram_tensor("ident", [128, 128], f16, kind="ExternalInput")
    wa_d = nc.dram_tensor("wa", [128, 5 * 2560], f16, kind="ExternalInput")
    wb_d = nc.dram_tensor("wb", [128, 17 * 1024], f16, kind="ExternalInput")
    wf_d = nc.dram_tensor("wf", [128, 5 * 640], f16, kind="ExternalInput")
    sabf_d = nc.dram_tensor("sabf", [16, 3], f32, kind="ExternalInput")  # sa,sb,sf

    xl1_sh = nc.dram_tensor("xl1_sh", [SHARD, FT], f16)
    xl2_sh = nc.dram_tensor("xl2_sh", [SHARD, FT], f16)
    xl1_tb = nc.dram_tensor("xl1_tb", [N, FT], f16, addr_space="Shared")
    xl2_tb = nc.dram_tensor("xl2_tb", [N, FT], f16, addr_space="Shared")
    zout = nc.dram_tensor("zout", [GPC, FT + 1], f32, kind="ExternalOutput")
    gmout = nc.dram_tensor("gmout", [GPC, FT + 1], f32, kind="ExternalOutput")

    with tile.TileContext(nc, num_cores=NCORES) as tc:
        import contextlib
        est = contextlib.ExitStack()
        with est:
            nc.gpsimd.load_library(gpsimd_mlp)
            nregs = {n: nc.gpsimd.to_reg(n) for n in NREG}
            cpool = est.enter_context(tc.tile_pool(name="consts", bufs=1))
            wpool = est.enter_context(tc.tile_pool(name="wmlp", bufs=1))
            xrp = est.enter_context(tc.tile_pool(name="xr", bufs=1))
            h1p = est.enter_context(tc.tile_pool(name="h1", bufs=1))
            sbp = est.enter_context(tc.tile_pool(name="stream", bufs=2))
            smp = est.enter_context(tc.tile_pool(name="small", bufs=2))
            msb = est.enter_context(tc.tile_pool(name="mstream", bufs=2))
            psb = est.enter_context(tc.tile_pool(name="psb", bufs=5, space="PSUM"))
            pss = est.enter_context(tc.tile_pool(name="pss", bufs=2, space="PSUM"))
            pgmp = est.enter_context(tc.tile_pool(name="pgm", bufs=1, space="PSUM"))

            # ---- consts ----
            w1l_s = cpool.tile([128, 5 * FT], f16, name="w1l_s")
            nc.sync.dma_start(w1l_s[:], w1l[:])
            w1r_s = cpool.tile([128, 5 * FT], f16, name="w1r_s")
            nc.sync.dma_start(w1r_s[:], w1r[:])
            w2l_s = cpool.tile([128, 4 * FT], f16, name="w2l_s")
            nc.sync.dma_start(w2l_s[:], w2l[:])
            w2r_s = cpool.tile([128, 4 * FT], f16, name="w2r_s")
            nc.sync.dma_start(w2r_s[:], w2r[:])
            b2la_s = cpool.tile([1, FT], f16, name="b2la_s")
            nc.sync.dma_start(b2la_s[:], b2r_d[0:1, :])
            b2ra_s = cpool.tile([1, FT], f16, name="b2ra_s")
            nc.sync.dma_start(b2ra_s[:], b2r_d[1:2, :])
            af1_s = cpool.tile([128, 2 * FT], f16, name="af1_s")
            nc.sync.dma_start(af1_s[:], af1_d[:])
            af2_s = cpool.tile([128, 2 * FT], f16, name="af2_s")
            nc.sync.dma_start(af2_s[:], af2_d[:])
            b1f_s = cpool.tile([128, FT], f32, name="b1f_s")
            nc.sync.dma_start(b1f_s[:], b1f_d[:])
            b2f_s = cpool.tile([128, FT], f32, name="b2f_s")
            nc.sync.dma_start(b2f_s[:], b2f_d[:])
            ident_s = cpool.tile([128, 128], f16, name="ident_s")
            nc.sync.dma_start(ident_s[:], ident_d[:])
            ecols_s = cpool.tile([128, NT * GPC], f16, name="ecols_s")
            nc.sync.dma_start(ecols_s[:], ecols_d[:])
            ones1 = cpool.tile([1, FT], f16, name="ones1")
            nc.vector.memset(ones1[:], 1.0)
            sabf_s = cpool.tile([16, 3], f32, name="sabf_s")
            nc.sync.dma_start(sabf_s[:], sabf_d[:])
            esc = cpool.tile([16, 3], f32, name="esc")
            nc.scalar.activation(esc[:], sabf_s[:], AF.Exp)
            z0p = cpool.tile([16, 640], f16, name="z0p")
            nc.vector.memset(z0p[:], 0.0)
            nc.vector.memset(z0p[:, 513:514], 1.0)
            onescol = cpool.tile([128, 1], f16, name="onescol")
            nc.vector.memset(onescol[:], 1.0)
            tacc = cpool.tile([128, NT], f32, name="tacc")
            alph = cpool.tile([128, 1], f32, name="alph")
            nc.vector.memset(alph[:], LEAK)
            nege = cpool.tile([128, 1], f32, name="nege")
            nc.vector.memset(nege[:], -2.0)
            # MLP weights rotate through one 35KB buffer: wa early (overlaps
            # the GNN), wb/wf reload behind each llin stage.
            wa_s = wpool.tile([128, 17 * 1024], f16, tag="w", name="wa_s", bufs=1)
            nc.sync.dma_start(wa_s[:, 0:5 * 2560], wa_d[:])
            wa_v = wa_s[:, 0:5 * 2560].rearrange("p (k n) -> p k n", k=5)

            def nsqrt(out_ap, x_ap, pool, pfx):
                """out = sqrt(x), Newton-refined (ACT sqrt LUT is ~4e-3)."""
                y0 = pool.tile(list(x_ap.shape), f32, tag="nsq", name=pfx + "y0",
                               bufs=6)
                nc.scalar.activation(y0[:], x_ap, AF.Sqrt)
                r0 = pool.tile(list(x_ap.shape), f32, tag="nsq", name=pfx + "r0",
                               bufs=6)
                nc.vector.reciprocal(r0[:], y0[:])
                nc.vector.tensor_tensor(out=r0[:], in0=x_ap, in1=r0[:],
                                        op=ALU.mult)
                nc.vector.tensor_tensor(out=y0[:], in0=y0[:], in1=r0[:],
                                        op=ALU.add)
                nc.vector.tensor_scalar_mul(out_ap, y0[:], 0.5)

            xr_s = xrp.tile([128, NT * FT], f16, name="xr_s")       # resident xr
            h1_s = h1p.tile([128, NT * FT], f16, name="h1_s")       # resident h1
            h1pre = h1p.tile([128, NT * FT], f16, name="h1pre")     # pre-gelu

            def allgather(g, sh, tb):
                # table rows are chunk-major (g, k, r): each chunk's gathered
                # output is one contiguous [8*CR, FT] slice (BIR requires it)
                nc.gpsimd.collective_compute(
                    "AllGather", ALU.bypass, replica_groups=groups,
                    ins=[sh[g * CR:(g + 1) * CR, :]],
                    outs=[tb[g * NCORES * CR:(g + 1) * NCORES * CR, :]])

            def transform1():
                xTa_v = xTa[:].rearrange("p (k n) -> p k n", k=5)
                w1l_v = w1l_s[:].rearrange("p (k n) -> p k n", k=5)
                w1r_v = w1r_s[:].rearrange("p (k n) -> p k n", k=5)
                for t in range(NT):
                    xt = smp.tile([128, 5 * 128], f16, tag="xTa_t", name="xt",
                                  bufs=3)
                    nc.sync.dma_start(
                        xt[:].rearrange("p (k n) -> p k n", k=5),
                        xTa_v[:, :, t * 128:(t + 1) * 128])
                    xt_v = xt[:].rearrange("p (k n) -> p k n", k=5)
                    pl = psb.tile([128, FT], f32, tag="pbig", name="pl")
                    pr = psb.tile([128, FT], f32, tag="pbig", name="pr")
                    for kc in range(5):
                        nc.tensor.matmul(pl[:], lhsT=xt_v[:, kc, :],
                                         rhs=w1l_v[:, kc, :],
                                         start=(kc == 0), stop=(kc == 4))
                        nc.tensor.matmul(pr[:], lhsT=xt_v[:, kc, :],
                                         rhs=w1r_v[:, kc, :],
                                         start=(kc == 0), stop=(kc == 4))
                    xlt = smp.tile([128, FT], f16, tag="xlt", name="xlt")
                    nc.scalar.activation(xlt[:], pl[:], AF.Copy)
                    nc.scalar.activation(xr_s[:, t * FT:(t + 1) * FT], pr[:],
                                         AF.Copy)
                    nc.sync.dma_start(xl1_sh[t * 128:(t + 1) * 128, :], xlt[:])
                    if (t + 1) % (NT // NCHUNK) == 0:
                        allgather(t // (NT // NCHUNK), xl1_sh, xl1_tb)

            def transform2():
                w2l_v = w2l_s[:].rearrange("p (k n) -> p k n", k=4)
                w2r_v = w2r_s[:].rearrange("p (k n) -> p k n", k=4)
                for t in range(NT):
                    h1t = h1_s[:].rearrange("p (t n) -> p t n", t=NT)[:, t, :]
                    h1T = smp.tile([128, 4 * 128], f16, tag="h1T", name="h1T")
                    for fc in range(4):
                        pt = pss.tile([128, 128], f16, tag="pe", name="pt",
                                      bufs=1)
                        nc.tensor.transpose(pt[:], h1t[:, fc * 128:(fc + 1) * 128],
                                            ident_s[:])
                        nc.scalar.activation(h1T[:, fc * 128:(fc + 1) * 128],
                                             pt[:], AF.Copy)
                    pl = psb.tile([128, FT], f32, tag="pbig", name="pl2")
                    pr = psb.tile([128, FT], f32, tag="pbig", name="pr2")
                    h1T_v = h1T[:].rearrange("p (k n) -> p k n", k=4)
                    for kc in range(4):
                        nc.tensor.matmul(pl[:], lhsT=h1T_v[:, kc, :],
                                         rhs=w2l_v[:, kc, :],
                                         start=(kc == 0), stop=False)
                        nc.tensor.matmul(pr[:], lhsT=h1T_v[:, kc, :],
                                         rhs=w2r_v[:, kc, :],
                                         start=(kc == 0), stop=False)
                    nc.tensor.matmul(pl[:], lhsT=ones1[:, 0:128], rhs=b2la_s[:],
                                     start=False, stop=True)
                    nc.tensor.matmul(pr[:], lhsT=ones1[:, 0:128], rhs=b2ra_s[:],
                                     start=False, stop=True)
                    xlt = smp.tile([128, FT], f16, tag="xlt", name="xlt2")
                    nc.scalar.activation(xlt[:], pl[:], AF.Copy)
                    nc.scalar.activation(xr_s[:, t * FT:(t + 1) * FT], pr[:],
                                         AF.Copy)
                    nc.sync.dma_start(xl2_sh[t * 128:(t + 1) * 128, :], xlt[:])
                    if (t + 1) % (NT // NCHUNK) == 0:
                        allgather(t // (NT // NCHUNK), xl2_sh, xl2_tb)

            # one 2-bank psum tile: centroid sums (cols 0:513, layer2) and the
            # per-tile softmax denominator (cols 624:628) share it
            pgm = pgmp.tile([128, 640], f32, name="pgm")

            def edge_layer(layer, table, af_s):
                """GATv2 message passing; writes h1_s (layer1) or, for layer2,
                the h2 epilogue + centroid accumulation + z0 extraction."""
                for t in range(NT):
                    NJt = njt[t]
                    idxt = smp.tile([128, LP // 16], i16, tag="idxt",
                                    name="idxt", bufs=4)
                    nc.sync.dma_start(idxt[:, 0:NJt * 8],
                                      idx_d[t, :, 0:NJt * 8])
                    sdtt = sbp.tile([128, LP], f16, tag="sdtt", name="sdtt")
                    nc.sync.dma_start(sdtt[:, 0:NJt * 128],
                                      sdt_d[t, :, 0:NJt * 128])
                    sjt = sbp.tile([128, NJ * 128], EXD, tag="sjt", name="sjt")
                    nc.sync.dma_start(sjt[:, 0:NJt * 128],
                                      sj_d[t, :, 0:NJt * 128])
                    sj_v = sjt[:].rearrange("p (j d) -> p j d", j=NJ)

                    xlg = sbp.tile([128, NJ * FT], f16, tag="xlg", name="xlg")
                    xlg_w = xlg[:].rearrange("p (j n) -> p j n", j=NJ)
                    for (o, n) in SCT[t]:
                        nc.gpsimd.dma_gather(
                            xlg_w[:, o // 128:(o + n) // 128, :], table[:],
                            idxt[:, o // 16:(o + n) // 16], n, nregs[n], FT)
                    xlg_v = xlg[:].rearrange("p (j n) -> p j n", j=NJ)

                    xr_t = xr_s[:].rearrange("p (t n) -> p t n", t=NT)[:, t, :]
                    logit = smp.tile([128, NJ * HEADS], f32, tag="logit",
                                     name="logit")
                    PAIRS = [(jb, min(2, NJt - jb)) for jb in range(0, NJt, 2)]
                    for jb in range(NJt):
                        ps2 = psb.tile([128, FT], f32, tag="pbig", name="ps2")
                        nc.tensor.matmul(ps2[:],
                                         lhsT=sdtt[:, jb * 128:(jb + 1) * 128],
                                         rhs=xr_t, start=True, stop=False)
                        nc.tensor.matmul(ps2[:], lhsT=ident_s[:],
                                         rhs=xlg_v[:, jb, :],
                                         start=False, stop=True)
                        lr = smp.tile([128, FT], f16, tag="lr", name="lr",
                                      bufs=3)
                        nc.scalar.activation(lr[:], ps2[:], AF.Prelu,
                                             alpha=alph[:])
                        scr = smp.tile([128, FT], f16, tag="scr", name="scr",
                                       bufs=2)
                        for h in range(HEADS):
                            nc.vector.scalar_tensor_tensor(
                                out=scr[:, h * C:(h + 1) * C],
                                in0=lr[:, h * C:(h + 1) * C], scalar=1.0,
                                in1=af_s[:, h * C:(h + 1) * C],
                                op0=ALU.mult, op1=ALU.mult,
                                accum_out=logit[:, jb * HEADS + h:
                                                jb * HEADS + h + 1])
                    exf = smp.tile([128, NJ * HEADS], f32, tag="exf", name="exf")
                    nc.scalar.activation(exf[:, 0:NJt * HEADS],
                                         logit[:, 0:NJt * HEADS], AF.Exp,
                                         bias=nege[:])
                    ex = smp.tile([128, NJ * HEADS], EXD, tag="ex", name="ex")
                    nc.scalar.activation(ex[:, 0:NJt * HEADS],
                                         exf[:, 0:NJt * HEADS], AF.Copy)
                    ex_v = ex[:].rearrange("p (j h) -> p j h", j=NJ)
                    pden = pgm[:, 624:628]
                    pagg = psb.tile([128, FT], f32, tag="pbig", name="pagg")
                    for (jb, w) in PAIRS:
                        wt2 = smp.tile([128, 2 * FT], EXD, tag="wt", name="wt",
                                       bufs=2)
                        nc.vector.tensor_tensor(
                            out=wt2[:, 0:w * FT].rearrange(
                                "p (j h c) -> p j h c", j=w, h=HEADS),
                            in0=xlg_v[:, jb:jb + w, :].rearrange(
                                "p j (h c) -> p j h c", h=HEADS),
                            in1=ex_v[:, jb:jb + w, :].broadcast_to(
                                [128, w, HEADS, C]),
                            op=ALU.mult)
                        for sub in range(w):
                            j = jb + sub
                            nc.tensor.matmul(pden, lhsT=sj_v[:, j, :],
                                             rhs=ex_v[:, j, :],
                                             start=(j == 0),
                                             stop=(j == NJt - 1))
                            nc.tensor.matmul(pagg[:], lhsT=sj_v[:, j, :],
                                             rhs=wt2[:, sub * FT:
                                                     (sub + 1) * FT],
                                             start=(j == 0),
                                             stop=(j == NJt - 1))
                    rden = smp.tile([128, HEADS], f32, tag="rden", name="rden")
                    nc.vector.reciprocal(rden[:], pden)
                    # epilogue: out = pagg*rden (per head) + bias (on gpsimd)
                    if layer == 1:
                        for h in range(HEADS):
                            nc.vector.scalar_tensor_tensor(
                                out=h1pre[:, t * FT + h * C:t * FT + (h + 1) * C],
                                in0=pagg[:, h * C:(h + 1) * C],
                                scalar=rden[:, h:h + 1],
                                in1=b1f_s[:, h * C:(h + 1) * C],
                                op0=ALU.mult, op1=ALU.add)
                        if (t + 1) % (NT // NCHUNK) == 0:
                            g = t // (NT // NCHUNK)
                            for tg in range(g * (NT // NCHUNK), (g + 1) *
                                            (NT // NCHUNK)):
                                nc.scalar.activation(
                                    h1_s[:, tg * FT:(tg + 1) * FT],
                                    h1pre[:, tg * FT:(tg + 1) * FT], AF.Gelu)
                    else:
                        h2sp = smp.tile([128, FT], f32, tag="h2sp", name="h2sp")
                        for h in range(HEADS):
                            nc.vector.scalar_tensor_tensor(
                                out=h2sp[:, h * C:(h + 1) * C],
                                in0=pagg[:, h * C:(h + 1) * C],
                                scalar=rden[:, h:h + 1],
                                in1=b2f_s[:, h * C:(h + 1) * C],
                                op0=ALU.mult, op1=ALU.add)
                        sqj = smp.tile([128, FT], f16, tag="sqj", name="sqj")
                        nc.vector.scalar_tensor_tensor(
                            out=sqj[:], in0=h2sp[:], scalar=1.0, in1=h2sp[:],
                            op0=ALU.mult, op1=ALU.mult,
                            accum_out=tacc[:, t:t + 1])
                        h2c = smp.tile([128, FT], f16, tag="h2c", name="h2c")
                        nc.scalar.activation(h2c[:], h2sp[:], AF.Copy)
                        ec = ecols_s[:, t * GPC:(t + 1) * GPC]
                        nc.tensor.matmul(pgm[:GPC, 0:FT], lhsT=ec, rhs=h2c[:],
                                         start=(t == 0), stop=(t == NT - 1))
                        nc.sync.dma_start(z0p[t:t + 1, 1:FT + 1], h2c[0:1, :])
                if layer == 2:
                    # batched time coordinate: t = sqrt(1 + |s|^2) for all tiles
                    nc.vector.tensor_scalar_add(tacc[:], tacc[:], 1.0)
                    tsq = smp.tile([128, NT], f32, tag="tsq", name="tsq")
                    nsqrt(tsq[:], tacc[:], smp, "t_")
                    tc16 = smp.tile([128, NT], f16, tag="tc16", name="tc16")
                    nc.vector.tensor_copy(tc16[:], tsq[:])
                    nc.tensor.matmul(pgm[:GPC, FT:FT + 1], lhsT=tc16[:],
                                     rhs=onescol[:], start=True, stop=True)
                    # z0 time coord recomputed from its (f16) space part
                    zsqj = msb.tile([16, FT], f16, tag="zsqj", name="zsqj")
                    zta = msb.tile([16, 1], f32, tag="t1", name="zta", bufs=8)
                    nc.vector.scalar_tensor_tensor(
                        out=zsqj[:], in0=z0p[:, 1:FT + 1], scalar=1.0,
                        in1=z0p[:, 1:FT + 1], op0=ALU.mult, op1=ALU.mult,
                        accum_out=zta[:])
                    nc.vector.tensor_scalar_add(zta[:], zta[:], 1.0)
                    ztb = msb.tile([16, 1], f32, tag="t1", name="ztb", bufs=8)
                    nsqrt(ztb[:], zta[:], msb, "zt_")
                    nc.scalar.activation(z0p[:, 0:1], ztb[:], AF.Copy)
                    return pgm

            transform1()
            edge_layer(1, xl1_tb, af1_s)
            transform2()
            pgm = edge_layer(2, xl2_tb, af2_s)

            # -------- centroid epilogue (pgm layout: [space(512) | time]) ----
            sums = smp.tile([GPC, FT + 1], f32, tag="sums", name="sums")
            nc.scalar.activation(sums[:], pgm[:GPC, 0:FT + 1], AF.Copy)
            sqgj = smp.tile([GPC, FT], f16, tag="sqgj", name="sqgj")
            sa_ = smp.tile([GPC, 1], f32, tag="sacc", name="sa_")
            nc.vector.scalar_tensor_tensor(
                out=sqgj[:], in0=sums[:, 0:FT], scalar=1.0, in1=sums[:, 0:FT],
                op0=ALU.mult, op1=ALU.mult, accum_out=sa_[:])
            innr = smp.tile([GPC, 1], f32, tag="in1", name="innr")
            nc.vector.tensor_tensor(out=innr[:], in0=sums[:, FT:FT + 1],
                                    in1=sums[:, FT:FT + 1], op=ALU.mult)
            nc.vector.tensor_tensor(out=innr[:], in0=innr[:], in1=sa_[:],
                                    op=ALU.subtract)
            nc.vector.tensor_scalar_max(innr[:], innr[:], 1e-8 * (N // B) ** 2)
            rt = smp.tile([GPC, 1], f32, tag="in1", name="rt")
            nsqrt(rt[:], innr[:], smp, "g_")
            nc.vector.reciprocal(rt[:], rt[:])
            gmt = smp.tile([GPC, FT + 1], f32, tag="sums", name="gmt")
            nc.scalar.activation(gmt[:, 0:1], sums[:, FT:FT + 1], AF.Copy,
                                 scale=rt[:])
            nc.scalar.activation(gmt[:, 1:FT + 1], sums[:, 0:FT], AF.Copy,
                                 scale=rt[:])
            nc.sync.dma_start(gmout[:], gmt[:])

            # ---------------- Lorentz MLP on z0 [16, 513] ----------------
            def trans_blocks(zp, kb):
                """zp [16, kb*128] f16 -> zT [128, kb*16] f16 via PE."""
                zT = msb.tile([128, 17 * 16], f16, tag="zT", name="zT")
                for k in range(kb):
                    pt = pss.tile([128, 128], f16, tag="pe", name="ptm",
                                  bufs=1)
                    nc.tensor.transpose(pt[:, 0:16], zp[:, k * 128:(k + 1) * 128],
                                        ident_s[:16, :16])
                    nc.scalar.activation(zT[:, k * 16:(k + 1) * 16], pt[:, 0:16],
                                         AF.Copy)
                return zT

            def mm_thin(zT, kb, w_v, ncols):
                """out [16, ncols] f32 = zT.T @ w; w_v view [128, kb, ncols]."""
                out = msb.tile([16, 2560], f32, tag="mlpo", name="out", bufs=1)
                zT_v = zT[:].rearrange("p (k n) -> p k n", k=17)
                for o in range(0, ncols, 512):
                    n = min(512, ncols - o)
                    pm = psb.tile([128, FT], f32, tag="pbig", name="pm")
                    for k in range(kb):
                        nc.tensor.matmul(pm[:16, :n], lhsT=zT_v[:, k, :16],
                                         rhs=w_v[:, k, o:o + n],
                                         start=(k == 0), stop=(k == kb - 1))
                    nc.scalar.activation(out[:, o:o + n], pm[:16, :n], AF.Copy)
                return out

            def llin_post(zz, kout, esc_idx):
                """returns (t1, r_) for zz [16, ncols>=kout] f32."""
                t1 = msb.tile([16, 1], f32, tag="t1", name="t1", bufs=8)
                nc.scalar.activation(t1[:], zz[:, 0:1], AF.Sigmoid)
                nc.vector.tensor_scalar(
                    out=t1[:], in0=t1[:],
                    scalar1=esc[:, esc_idx:esc_idx + 1],
                    scalar2=1.1, op0=ALU.mult, op1=ALU.add)
                sq = msb.tile([16, 2048], f32, tag="msq", name="sq", bufs=1)
                ac = msb.tile([16, 1], f32, tag="t1", name="ac", bufs=8)
                nc.scalar.activation(sq[:, :kout - 1], zz[:, 1:kout], AF.Square,
                                     accum_out=ac[:])
                nc.vector.tensor_scalar_max(ac[:], ac[:], 1e-8)
                r_ = msb.tile([16, 1], f32, tag="t1", name="r_", bufs=8)
                nc.vector.reciprocal(r_[:], ac[:])
                t2 = msb.tile([16, 1], f32, tag="t1", name="t2", bufs=8)
                nc.vector.tensor_tensor(out=t2[:], in0=t1[:], in1=t1[:],
                                        op=ALU.mult)
                nc.vector.tensor_scalar_add(t2[:], t2[:], -1.0)
                nc.vector.tensor_tensor(out=r_[:], in0=r_[:], in1=t2[:],
                                        op=ALU.mult)
                nsqrt(r_[:], r_[:], msb, "m_")
                return t1, r_

            # llin-a: z0p [16, 640] -> zA [16, 2560]
            zT = trans_blocks(z0p, 5)
            zA = mm_thin(zT, 5, wa_v, 2560)
            t1, r1 = llin_post(zA, 2049, 0)
            wb_s = wpool.tile([128, 17 * 1024], f16, tag="w", name="wb_s",
                              bufs=1)
            nc.sync.dma_start(wb_s[:], wb_d[:])
            wb_v = wb_s[:].rearrange("p (k n) -> p k n", k=17)
            # z1 = add_time(gelu(sp*r1)): gelu with scale=r1
            z1p = msb.tile([16, 17 * 128], f16, tag="z1p", name="z1p", bufs=1)
            nc.vector.memset(z1p[:], 0.0)
            nc.scalar.activation(z1p[:, 1:2049], zA[:, 1:2049], AF.Gelu,
                                 scale=r1[:])
            sqz = msb.tile([16, 2048], f32, tag="msq", name="sqz", bufs=1)
            az = msb.tile([16, 1], f32, tag="t1", name="az", bufs=8)
            nc.scalar.activation(sqz[:], z1p[:, 1:2049], AF.Square,
                                 accum_out=az[:])
            az1 = msb.tile([16, 1], f32, tag="t1", name="az1", bufs=8)
            nc.scalar.activation(az1[:], az[:], AF.Identity, bias=1.0)
            nsqrt(z1p[:, 0:1], az1[:], msb, "z_")
            nc.vector.memset(z1p[:, 2049:2050], 1.0)
            # llin-b: [16, 2049] -> [16, 513]
            zTb = trans_blocks(z1p, 17)
            zB = mm_thin(zTb, 17, wb_v, 1024)
            t3, r3 = llin_post(zB, 513, 1)
            wf_s = wpool.tile([128, 17 * 1024], f16, tag="w", name="wf_s",
                              bufs=1)
            nc.sync.dma_start(wf_s[:, 0:5 * 640], wf_d[:])
            wf_v = wf_s[:, 0:5 * 640].rearrange("p (k n) -> p k n", k=5)
            z2p = msb.tile([16, 640], f16, tag="z2p", name="z2p", bufs=1)
            nc.vector.memset(z2p[:], 0.0)
            nc.scalar.activation(z2p[:, 0:1], t3[:], AF.Copy)
            nc.scalar.activation(z2p[:, 1:513], zB[:, 1:513], AF.Copy,
                                 scale=r3[:])
            nc.vector.memset(z2p[:, 513:514], 1.0)
            # llin-f: [16, 513] -> [16, 513]
            zTf = trans_blocks(z2p, 5)
            zF = mm_thin(zTf, 5, wf_v, 640)
            t4, r4 = llin_post(zF, 513, 2)
            zfin = msb.tile([16, 640], f32, tag="zfin", name="zfin", bufs=1)
            nc.scalar.activation(zfin[:, 0:1], t4[:], AF.Copy)
            nc.scalar.activation(zfin[:, 1:513], zF[:, 1:513], AF.Copy,
                                 scale=r4[:])
            nc.sync.dma_start(zout[:], zfin[:, 0:FT + 1])

    nc.compile()
    return nc


# ============================ host entry ============================

EX_DTYPE = "float16"    # logits are small; exp shifted by -2


def _make_inmaps(inputs):
    x = np.asarray(inputs["x"], np.float32)
    edge_index = np.asarray(inputs["edge_index"])
    srcs, sdt, sjt, idx, LP, NJ, njt = _prep_edges(edge_index)

    f16 = np.float16
    exd_np = ml_dtypes.bfloat16 if EX_DTYPE == "bfloat16" else np.float16

    # ---- shared (replicated) host arrays ----
    def aug5(W, b):
        return _interleave_k(_aug_w(np.asarray(W, np.float32),
                                    np.asarray(b, np.float32), 640), 5)

    w1l_h = aug5(inputs["Wl1"], inputs["bl1"]).astype(f16).reshape(128, 5 * FT)
    w1r_h = aug5(inputs["Wr1"], inputs["br1"]).astype(f16).reshape(128, 5 * FT)
    w2l_h = _interleave_k(np.asarray(inputs["Wl2"], np.float32), 4
                          ).astype(f16).reshape(128, 4 * FT)
    w2r_h = _interleave_k(np.asarray(inputs["Wr2"], np.float32), 4
                          ).astype(f16).reshape(128, 4 * FT)
    b2rows = np.stack([np.asarray(inputs["bl2"]),
                       np.asarray(inputs["br2"])]).astype(f16)

    def amat(att):
        att = np.asarray(att, np.float32).reshape(1, HEADS * C)
        return np.tile(att, (128, 2)).astype(f16)

    af1_h, af2_h = amat(inputs["att1"]), amat(inputs["att2"])
    b1full = np.tile(np.asarray(inputs["bias1"], np.float32)[None, :], (128, 1))
    b2full = np.tile(np.asarray(inputs["bias2"], np.float32)[None, :], (128, 1))
    ecols = np.zeros((128, NT * GPC), f16)
    for t in range(NT):
        ecols[:, t * GPC + t] = 1.0
    ident = np.eye(128, dtype=f16)
    wa_h = _interleave_k(_aug_w(np.asarray(inputs["Wa"], np.float32),
                                np.asarray(inputs["ba"], np.float32), 640, 2560),
                         5).astype(f16).reshape(128, 5 * 2560)
    wb_h = _interleave_k(_aug_w(np.asarray(inputs["Wb"], np.float32),
                                np.asarray(inputs["bb"], np.float32), 17 * 128,
                                1024), 17).astype(f16).reshape(128, 17 * 1024)
    wf_h = _interleave_k(_aug_w(np.asarray(inputs["Wf"], np.float32),
                                np.asarray(inputs["bf"], np.float32), 640, 640),
                         5).astype(f16).reshape(128, 5 * 640)
    sabf = np.tile(np.array([[float(inputs["sa"]), float(inputs["sb"]),
                              float(inputs["sf"])]], np.float32), (16, 1))

    in_maps = []
    for k in range(NCORES):
        sl = slice(k * SHARD, (k + 1) * SHARD)
        xs = x[sl, 1:]                       # [2048, 512]
        xT = np.zeros((640, SHARD), np.float32)
        xT[:FT] = xs.T
        xT[FT] = 1.0
        xTa_h = _interleave_k(xT, 5).astype(f16).reshape(128, 5 * SHARD)
        tsl = slice(k * NT, (k + 1) * NT)
        in_maps.append({
            "xTa": xTa_h, "w1l": w1l_h, "w1r": w1r_h, "w2l": w2l_h,
            "w2r": w2r_h, "b2rows": b2rows, "attf1": af1_h, "attf2": af2_h,
            "b1full": b1full, "b2full": b2full,
            "sdt": sdt[tsl].astype(f16),
            "sj": np.ascontiguousarray(
                sjt[tsl].reshape(NT, NJ, 128, 128).transpose(0, 2, 1, 3)
            ).reshape(NT, 128, NJ * 128).astype(exd_np),
            "idx": idx[tsl],
            "ecols": ecols, "ident": ident,
            "wa": wa_h, "wb": wb_h, "wf": wf_h, "sabf": sabf,
        })

    return in_maps, LP, NJ, njt


_last_exec_ns = None


def kernel(**inputs):
    global _last_exec_ns
    in_maps, LP, NJ, njt = _make_inmaps(inputs)
    key = (LP, EX_DTYPE, tuple(njt))
    if key not in _cache:
        _cache[key] = _build(LP, NJ, EX_DTYPE, njt)
    nc = _cache[key]
    from concourse.bass_utils import run_bass_kernel_spmd
    res = run_bass_kernel_spmd(nc, in_maps, list(range(NCORES)))
    _last_exec_ns = res.exec_time_ns
    kernel._last_res = res
    z = np.concatenate([np.asarray(r["zout"]) for r in res.results], 0)
    gm = np.concatenate([np.asarray(r["gmout"]) for r in res.results], 0)
    return z.astype(np.float32), gm.astype(np.float32)
